# revision 6
# baseline (speedup 1.0000x reference)
"""Cross-attention + output projection + residual + GroupNorm on 8 NeuronCores.

Problem (hardcoded): B=4, C=256, H=W=48 (N=2304 pixels), 4 heads x 64 dim,
GroupNorm with 32 groups of 8 channels, eps=1e-5.

Sharding: 2 cores per batch element; each core handles one half of the
query pixels (1152) for all 4 heads.  K/V are computed for the full pixel
range on both cores of a pair (duplicated, cheap).  The only cross-core
communication is a 2KB AllReduce of per-channel (sum, sumsq) GroupNorm
partial statistics between the two cores of each pair.

v2 design (vs. the 169us baseline whose ACT engine was the 103us wall):
  * softmax exp is split across TWO engines, whole-tile alternating:
      ACT:  ex = Exp(scale * s)           (LUT, (N+352)/1.2 ns)
      DVE:  ex = bits_fp16(int16(A*s+B))  (Schraudolph 2^x bit trick,
            A = 1024*log2(e)*scale, B = 15*1024 - 45; ~3% rel err, which
            lands ~1e-3 on the final output -- validated vs the oracle)
  * score matmuls are row-tiled: each head's 64-dim contraction runs as
    two concurrent 64-row PE tiles (partitions 0-63 / 64-127) fed by
    partition-swapped duplicates q2/k2 (made by SBUF->SBUF DMA).
  * AV accumulates within the head's own slot (lag 2) so no AV work is
    left after the last exp; packed [i-tile x (64+ones)] PSUM banks, the
    bank-first matmul uses start=True instead of a pre-memset.
  * V projection runs up front in the projection-phase PSUM pool.
  * tail: Wo/residual/stats chunked right behind the last AV; channel
    (sum,sumsq) AllReduce launched as early as possible; final affine on
    DVE/gpsimd interleaved with the output DMA.
"""

import sys

if "/opt/trn_rl_repo" not in sys.path:
    sys.path.insert(0, "/opt/trn_rl_repo")

import math

import ml_dtypes
import numpy as np

import concourse.bass as bass
import concourse.mybir as mybir
import concourse.tile as tile
from concourse import bacc
from concourse.bass_utils import run_bass_kernel_spmd

F32 = mybir.dt.float32
BF16 = mybir.dt.bfloat16
F16 = mybir.dt.float16
I16 = mybir.dt.int16
AF = mybir.ActivationFunctionType
ALU = mybir.AluOpType
AXF = mybir.AxisListType.X

B, C, HW = 4, 256, 2304
NH, HD = 4, 64
NHALF = HW // 2  # 1152 query pixels per core
NJT = HW // 128  # 18 key tiles of 128
NIT = NHALF // 128  # 9 query tiles of 128
SCALE = HD ** -0.5  # 0.125
GSIZE = 8  # channels per GroupNorm group
EPS = 1e-5
GN_COUNT = GSIZE * HW  # elements per group per batch (after pair AllReduce)

# Schraudolph fp16 exp: exp(SCALE*s) ~= bits_fp16(int16(EXP_A*s + EXP_B))
EXP_A = 1024.0 * math.log2(math.e) * SCALE
EXP_B = 15.0 * 1024.0 - 45.0

NA = 5  # query tiles in AV accumulator bank A (rest in bank B)
AV_LAG = 2  # AV for (h, jt) issues in slot position jt+AV_LAG

import os as _os

def _flag(name, default):
    v = _os.environ.get(name)
    return default if v is None else v == "1"

LDW_REUSE = _flag("CA_LDW_REUSE", True)  # skip LDWEIGHTS when lhsT repeats
USE_GPSIMD = _flag("CA_GPSIMD", True)  # gpsimd for xbo staging / affine
USE_DVE_EXP = _flag("CA_DVE_EXP", True)  # Schraudolph exp tiles on DVE
AV_NO_MEMSET = _flag("CA_AV_NOMEMSET", True)  # bank-first start=True trick

# whole-tile exp engine assignment: number of DVE tiles per head
_DVE_PER_HEAD = (6, 8, 9, 9)


def _dve_exp_set():
    s = set()
    for h, n in enumerate(_DVE_PER_HEAD):
        for i in range(n):
            s.add((h, int((i + 0.5) * NJT / n)))
    return s


DVE_EXP = _dve_exp_set() if USE_DVE_EXP else set()

_CACHE = {}


def _mm_slices(total, step=512):
    return [(s, min(s + step, total)) for s in range(0, total, step)]


def _finalize(nc):
    """compile() leaves 3+-wait Matmults that walrus rejects ("Too many sync
    wait commands" on the S3_LW struct); a second compile pass — run here via
    finalize() — splits them onto EventSemaphores.  Verify that it worked."""
    nc.compile()
    nc.finalize()
    for fn in nc.m.functions:
        for bb in fn.blocks:
            for inst in bb.instructions:
                si = inst.sync_info
                if isinstance(inst, mybir.InstMatmult) and si is not None:
                    assert len(si.on_wait or []) <= 2, (inst.name, si.on_wait)


def _build(dbg=False):
    nc = bacc.Bacc("TRN2", target_bir_lowering=False, debug=False, num_devices=8)

    xh_d = nc.dram_tensor("xh", [C, NHALF], F16, kind="ExternalInput").ap()
    ctx_d = nc.dram_tensor("ctx", [C, HW], F16, kind="ExternalInput").ap()
    w_d = {
        nm: nc.dram_tensor(nm, [C, C], F16, kind="ExternalInput").ap()
        for nm in ("wqT", "wkT", "wvT", "woT")
    }
    b_d = {
        nm: nc.dram_tensor(nm, [C, 1], F32, kind="ExternalInput").ap()
        for nm in ("bq", "bk", "bo", "gamma", "beta")
    }
    gsel_d = nc.dram_tensor("gsel", [128, 16], F32, kind="ExternalInput").ap()
    gselT_d = nc.dram_tensor("gselT", [16, 128], F32, kind="ExternalInput").ap()
    yh_d = nc.dram_tensor("yh", [C, NHALF], F32, kind="ExternalOutput").ap()
    if dbg:
        dbg_q = nc.dram_tensor("dbg_q", [C, NHALF], F16, kind="ExternalOutput").ap()
        dbg_k = nc.dram_tensor("dbg_k", [C, HW], F16, kind="ExternalOutput").ap()
        dbg_q2 = nc.dram_tensor("dbg_q2", [C, NHALF], F16, kind="ExternalOutput").ap()
        dbg_ex = nc.dram_tensor("dbg_ex", [128, 2, NHALF], F16, kind="ExternalOutput").ap()
        dbg_v = nc.dram_tensor("dbg_v", [128, NJT, NH * (HD + 1)], F16, kind="ExternalOutput").ap()
        dbg_ao = nc.dram_tensor("dbg_ao", [C, NHALF], F16, kind="ExternalOutput").ap()
        dbg_y = nc.dram_tensor("dbg_y", [C, NHALF], F32, kind="ExternalOutput").ap()
        dbg_st = nc.dram_tensor("dbg_st", [C, 2], F32, kind="ExternalOutput").ap()
    warm_d = nc.dram_tensor("warm_out", [16, 1], F32, kind="ExternalOutput").ap()

    with tile.TileContext(nc) as tc:
        with (
            tc.tile_pool(name="const", bufs=1) as const,
            tc.tile_pool(name="main", bufs=1) as main,
            tc.tile_pool(name="small", bufs=4) as small,
            tc.tile_pool(name="dram", bufs=2, space="DRAM") as dram,
        ):
            # ---- constants (DMA order = consumption order) ----
            w_sb = {}
            for nm in ("wqT", "wkT", "wvT", "woT"):
                w_sb[nm] = const.tile([128, 2, C], F16, tag=nm, name=nm)
            b_sb = {}
            for nm in ("bq", "bk", "bo", "gamma", "beta"):
                b_sb[nm] = const.tile([128, 2], F32, tag=nm, name=nm)

            xh_sb = main.tile([128, 2, NHALF], F16, tag="xh")
            nc.sync.dma_start(
                out=w_sb["wqT"], in_=w_d["wqT"].rearrange("(k p) o -> p k o", p=128)
            )
            nc.sync.dma_start(out=xh_sb, in_=xh_d.rearrange("(k p) i -> p k i", p=128))
            nc.sync.dma_start(
                out=w_sb["wkT"], in_=w_d["wkT"].rearrange("(k p) o -> p k o", p=128)
            )

            # warm the Exp ACT table while input DMAs run
            eps_sb = const.tile([16, 1], F32, tag="eps")
            nc.vector.memset(eps_sb, EPS)
            warm_e = small.tile([16, 1], F32, tag="warme", bufs=1)
            nc.scalar.activation(out=warm_e, in_=eps_sb, func=AF.Exp)

            # q/k in head-parity layout: partitions 0-63 = head 2g,
            # 64-127 = head 2g+1 (natural channel order); q2/k2 are the
            # partition-swapped duplicates for row-tiled score matmuls.
            q_sb = main.tile([128, 2, NHALF], F16, tag="q")
            q2_sb = main.tile([128, 2, NHALF], F16, tag="q2")
            k_sb = main.tile([128, 2, HW], F16, tag="k")
            k2_sb = main.tile([128, 2, HW], F16, tag="k2")
            vT_sb = main.tile([128, NJT, NH * (HD + 1)], F16, tag="vT")
            ao_sb = main.tile([128, 2, NHALF], F16, tag="ao")
            aon_sb = main.tile([128, 2, NIT, 128], F16, tag="aon")
            xbo_sb = main.tile([128, 2, NHALF], F32, tag="xbo")
            y_sb = main.tile([128, 2, NHALF], F32, tag="y")
            scr_sb = main.tile([128, NHALF], F16, tag="scr")

            # ones columns of vT (one per head, strided over j-tiles)
            for h in range(NH):
                c0 = h * (HD + 1) + HD
                (nc.gpsimd if USE_GPSIMD else nc.vector).memset(vT_sb[:, :, c0 : c0 + 1], 1.0)

            ctx_sb = main.tile([128, 2, HW], F16, tag="ctx")

            # ---- projections (fp16) + V, all in the pp PSUM pool ----
            with (
                tc.tile_pool(name="pp", bufs=2, space="PSUM") as pp,
            ):
                ctx_r = ctx_d.rearrange("(k p) j -> p k j", p=128)
                for jq in range(4):
                    s0, s1 = jq * (HW // 4), (jq + 1) * (HW // 4)
                    nc.sync.dma_start(out=ctx_sb[:, :, s0:s1], in_=ctx_r[:, :, s0:s1])

                nc.sync.dma_start(
                    out=w_sb["wvT"], in_=w_d["wvT"].rearrange("(k p) o -> p k o", p=128)
                )
                nc.sync.dma_start(
                    out=w_sb["woT"], in_=w_d["woT"].rearrange("(k p) o -> p k o", p=128)
                )
                for nm in ("bq", "bk", "bo", "gamma", "beta"):
                    nc.sync.dma_start(
                        out=b_sb[nm],
                        in_=b_d[nm].rearrange("(k p) one -> p (k one)", p=128),
                    )
                gsel_sb = const.tile([128, 16], F32, tag="gsel")
                nc.sync.dma_start(out=gsel_sb, in_=gsel_d)
                gselT_sb = const.tile([16, 128], F32, tag="gselT")
                nc.sync.dma_start(out=gselT_sb, in_=gselT_d)

                # Q: [o_grp 128, 1152] -> q_sb, bias add fused in the copy
                for g in range(2):
                    ps = pp.tile([128, NHALF], F32, tag="qk")
                    for kk in range(2):
                        lhsT = w_sb["wqT"][:, kk, g * 128 : (g + 1) * 128]
                        for s, e in _mm_slices(NHALF):
                            nc.tensor.matmul(
                                ps[:, s:e], lhsT, xh_sb[:, kk, s:e],
                                start=(kk == 0), stop=(kk == 1),
                            )
                    nc.scalar.activation(
                        out=q_sb[:, g, :], in_=ps, func=AF.Identity,
                        bias=b_sb["bq"][:, g : g + 1],
                    )
                # duplicate with swapped partition halves (DMA, off-engine)
                nc.sync.dma_start(out=q2_sb[0:64, :, :], in_=q_sb[64:128, :, :])
                nc.sync.dma_start(out=q2_sb[64:128, :, :], in_=q_sb[0:64, :, :])

                # K: [o_grp 128, 2304] in two j-halves -> k_sb
                for jh in range(2):
                    for g in range(2):
                        ps = pp.tile([128, NHALF], F32, tag="qk")
                        for kk in range(2):
                            lhsT = w_sb["wkT"][:, kk, g * 128 : (g + 1) * 128]
                            for s, e in _mm_slices(NHALF):
                                nc.tensor.matmul(
                                    ps[:, s:e], lhsT,
                                    ctx_sb[:, kk, jh * NHALF + s : jh * NHALF + e],
                                    start=(kk == 0), stop=(kk == 1),
                                )
                        nc.scalar.activation(
                            out=k_sb[:, g, jh * NHALF : (jh + 1) * NHALF],
                            in_=ps, func=AF.Identity,
                            bias=b_sb["bk"][:, g : g + 1],
                        )
                    nc.sync.dma_start(
                        out=k2_sb[0:64, :, jh * NHALF : (jh + 1) * NHALF],
                        in_=k_sb[64:128, :, jh * NHALF : (jh + 1) * NHALF],
                    )
                    nc.sync.dma_start(
                        out=k2_sb[64:128, :, jh * NHALF : (jh + 1) * NHALF],
                        in_=k_sb[0:64, :, jh * NHALF : (jh + 1) * NHALF],
                    )

                # V: vT[j, c] = (ctx.T @ WvT)[j, c] per j-tile (up front)
                for jt in range(NJT):
                    vp = pp.tile([128, C], F32, tag="vp")
                    for kk in range(2):
                        nc.tensor.matmul(
                            vp, ctx_sb[:, kk, jt * 128 : (jt + 1) * 128],
                            w_sb["wvT"][:, kk, :],
                            start=(kk == 0), stop=(kk == 1),
                        )
                    nc.vector.tensor_copy(
                        out=vT_sb[:, jt, :]
                        .rearrange("p (h e) -> p h e", e=HD + 1)[:, :, :HD],
                        in_=vp.rearrange("p (h d) -> p h d", d=HD),
                    )

                # residual + bo staged in fp32 (gpsimd, off the hot engines)
                xbo_eng = nc.gpsimd if USE_GPSIMD else nc.vector
                for g in range(2):
                    xbo_eng.tensor_scalar_add(
                        out=xbo_sb[:, g, :], in0=xh_sb[:, g, :],
                        scalar1=b_sb["bo"][:, g : g + 1],
                    )

            # ---- attention: per head, AV trails scores/exp by AV_LAG ----
            with (
                tc.tile_pool(name="expp", bufs=6) as expp,
                tc.tile_pool(name="scp", bufs=2, space="PSUM") as scp,
                tc.tile_pool(name="avp", bufs=1, space="PSUM") as avp,
            ):
                ex_tiles = {}
                av_banks = {}
                rdens = {}

                def issue_scores(h, jt):
                    g, ph = h // 2, h % 2
                    qlo, klo = (q_sb, k_sb) if ph == 0 else (q2_sb, k2_sb)
                    qhi, khi = (q2_sb, k2_sb) if ph == 0 else (q_sb, k_sb)
                    sc = scp.tile([128, NHALF], F32, tag="sc", name=f"sc{h}_{jt}")
                    j0, j1 = jt * 128, (jt + 1) * 128
                    m1 = nc.tensor.matmul(
                        sc[:, 0:512], klo[0:64, g, j0:j1], qlo[0:64, g, 0:512],
                        start=True, stop=True,
                    )
                    nc.tensor.matmul(
                        sc[:, 512:1024], khi[64:128, g, j0:j1],
                        qhi[64:128, g, 512:1024],
                        start=True, stop=True,
                    )
                    m3 = nc.tensor.matmul(
                        sc[:, 1024:1152], klo[0:64, g, j0:j1],
                        qlo[0:64, g, 1024:1152],
                        start=True, stop=True,
                    )
                    if LDW_REUSE:
                        m3.ins.ldweights = False  # same lhsT as m1
                    return sc

                def issue_exp(h, jt, sc):
                    ex = expp.tile([128, NHALF], F16, tag="exp", name=f"ex{h}_{jt}")
                    if (h, jt) in DVE_EXP:
                        nc.vector.tensor_scalar(
                            out=ex.bitcast(I16), in0=sc,
                            scalar1=EXP_A, scalar2=EXP_B,
                            op0=ALU.mult, op1=ALU.add,
                        )
                    else:
                        nc.scalar.activation(out=ex, in_=sc, func=AF.Exp,
                                             scale=SCALE)
                    ex_tiles[h, jt] = ex

                def issue_av(h, jt):
                    ava, avb = av_banks[h]
                    col = h * (HD + 1)
                    for it in range(NIT):
                        dst = (
                            ava[:, it * (HD + 1) : (it + 1) * (HD + 1)]
                            if it < NA
                            else avb[:, (it - NA) * (HD + 1) : (it - NA + 1) * (HD + 1)]
                        )
                        nc.tensor.matmul(
                            dst,
                            ex_tiles[h, jt][:, it * 128 : (it + 1) * 128],
                            vT_sb[:, jt, col : col + HD + 1],
                            start=(AV_NO_MEMSET and jt == 0 and it in (0, NA)),
                            stop=(jt == NJT - 1),
                            skip_group_check=True,
                        )
                    if jt == NJT - 1:
                        del ex_tiles[h, jt]  # allow pool rotation bookkeeping
                    else:
                        del ex_tiles[h, jt]

                def issue_norm(h, last):
                    """denominator reciprocal + normalize into aon."""
                    ava, avb = av_banks[h]
                    g, ph = h // 2, h % 2
                    rdena = small.tile([128, NA], F32, tag="rdena", bufs=2)
                    rdenb = small.tile([128, NIT - NA], F32, tag="rdenb", bufs=2)
                    nc.vector.reciprocal(out=rdena, in_=ava[:, HD :: HD + 1])
                    nc.vector.reciprocal(out=rdenb, in_=avb[:, HD :: HD + 1])
                    for it in range(NIT):
                        src = (
                            ava[:, it * (HD + 1) : it * (HD + 1) + HD]
                            if it < NA
                            else avb[:, (it - NA) * (HD + 1) : (it - NA) * (HD + 1) + HD]
                        )
                        rd = (
                            rdena[:, it : it + 1] if it < NA
                            else rdenb[:, it - NA : it - NA + 1]
                        )
                        out = aon_sb[:, g, it, ph * HD : (ph + 1) * HD]
                        nc.vector.tensor_scalar_mul(out=out, in0=src,
                                                    scalar1=rd)

                for h in range(NH):
                    av_banks[h] = (
                        avp.tile([128, NA * (HD + 1)], F32, tag="ava",
                                 name=f"ava{h}"),
                        avp.tile([128, (NIT - NA) * (HD + 1)], F32, tag="avb",
                                 name=f"avb{h}"),
                    )
                    if not AV_NO_MEMSET:
                        nc.vector.memset(av_banks[h][0], 0.0)
                        nc.vector.memset(av_banks[h][1], 0.0)
                    for jt in range(NJT):
                        sc = issue_scores(h, jt)
                        issue_exp(h, jt, sc)
                        if jt >= AV_LAG:
                            issue_av(h, jt - AV_LAG)
                    for jt in range(NJT - AV_LAG, NJT):
                        issue_av(h, jt)
                    issue_norm(h, last=(h == NH - 1))
                    if h % 2 == 1:  # both heads of the g-half staged
                        nc.sync.dma_start(
                            out=ao_sb[:, h // 2, :]
                            .rearrange("p (t i) -> p t i", i=128),
                            in_=aon_sb[:, h // 2, :, :],
                            transpose=True,
                        )

            if dbg:
                nc.sync.dma_start(out=dbg_q.rearrange("(k p) i -> p k i", p=128), in_=q_sb)
                nc.sync.dma_start(out=dbg_q2.rearrange("(k p) i -> p k i", p=128), in_=q2_sb)
                nc.sync.dma_start(out=dbg_k.rearrange("(k p) i -> p k i", p=128), in_=k_sb)
                nc.sync.dma_start(out=dbg_v, in_=vT_sb)
                nc.sync.dma_start(out=dbg_ao.rearrange("(k p) i -> p k i", p=128), in_=ao_sb)

            # Sqrt lives in a different ACT table set than Exp: warm the sqrt
            # table now (all exps done) so the GN-tail sqrt needs no load.
            # DMA to a real output so dead-code elimination keeps it.
            warm = small.tile([16, 1], F32, tag="warm", bufs=1)
            nc.scalar.sqrt(out=warm, in_=warm_e)
            nc.sync.dma_start(out=warm_d, in_=warm)

            # ---- output projection + residual + GroupNorm ----
            with (
                tc.tile_pool(name="wop", bufs=2, space="PSUM") as wop,
                tc.tile_pool(name="gnp", bufs=1, space="PSUM") as gnp,
            ):
                st4 = small.tile([128, 2, 2], F32, tag="st4", bufs=1)
                wps = []
                for kk in range(2):
                    for g in range(2):
                        if kk == 0:
                            wps.append(wop.tile([128, NHALF], F32, tag="wo",
                                                name=f"wo{g}"))
                        lhsT = w_sb["woT"][:, kk, g * 128 : (g + 1) * 128]
                        for s, e in _mm_slices(NHALF):
                            nc.tensor.matmul(wps[g][:, s:e], lhsT,
                                             ao_sb[:, kk, s:e],
                                             start=(kk == 0), stop=(kk == 1))
                for g in range(2):
                    nc.vector.tensor_tensor(
                        out=y_sb[:, g, :], in0=wps[g], in1=xbo_sb[:, g, :],
                        op=ALU.add,
                    )
                    nc.vector.reduce_sum(
                        out=st4[:, g, 0:1], in_=y_sb[:, g, :], axis=AXF
                    )
                    nc.scalar.activation(
                        out=scr_sb, in_=y_sb[:, g, :], func=AF.Square,
                        accum_out=st4[:, g, 1:2],
                    )

                # pair AllReduce of per-channel (sum, sumsq)
                gn_in = dram.tile([C, 2], F32, tag="gnin", bufs=1)
                gn_out = dram.tile([C, 2], F32, tag="gnout", bufs=1)
                nc.sync.dma_start(
                    out=gn_in.rearrange("(k p) s -> p k s", p=128), in_=st4
                )
                nc.gpsimd.collective_compute(
                    "AllReduce", ALU.add,
                    replica_groups=[[0, 1], [2, 3], [4, 5], [6, 7]],
                    ins=[gn_in.opt()], outs=[gn_out.opt()],
                )
                gs_sb = small.tile([128, 2, 2], F32, tag="gs", bufs=1)
                nc.sync.dma_start(
                    out=gs_sb, in_=gn_out.rearrange("(k p) s -> p k s", p=128)
                )

                # group totals via 0/1 selection matmul: [16 groups, (sum,sumsq)]
                gtot = small.tile([16, 2, 2], F32, tag="gtot", bufs=1)
                for kk in range(2):
                    gp = gnp.tile([16, 2], F32, tag="gp")
                    nc.tensor.matmul(gp, gsel_sb, gs_sb[:, kk, :],
                                     start=True, stop=True)
                    nc.vector.tensor_copy(out=gtot[:, kk, :], in_=gp)
                mean_g = gtot[:, :, 0]
                var_g = small.tile([16, 2], F32, tag="varg", bufs=1)
                nc.vector.tensor_copy(out=var_g, in_=gtot[:, :, 1])
                m2 = small.tile([16, 2], F32, tag="m2", bufs=1)
                nc.vector.tensor_mul(out=m2, in0=mean_g, in1=mean_g)
                nc.vector.tensor_tensor(out=var_g, in0=var_g, in1=m2,
                                        op=ALU.subtract)
                nc.scalar.activation(out=var_g, in_=var_g, func=AF.Sqrt,
                                     bias=eps_sb)
                nc.vector.reciprocal(out=var_g, in_=var_g)  # rstd [16, 2]

                # broadcast group stats to channels, fold gamma/beta into A,B
                mean_c = small.tile([128, 2], F32, tag="meanc", bufs=1)
                rstd_c = small.tile([128, 2], F32, tag="rstdc", bufs=1)
                for src, dst in ((mean_g, mean_c), (var_g, rstd_c)):
                    gp = gnp.tile([128, 2], F32, tag="gb")
                    nc.tensor.matmul(gp, gselT_sb, src, start=True, stop=True)
                    nc.vector.tensor_copy(out=dst, in_=gp)
                a_c = small.tile([128, 2], F32, tag="ac", bufs=1)
                b_c = small.tile([128, 2], F32, tag="bc", bufs=1)
                nc.vector.tensor_mul(out=a_c, in0=rstd_c, in1=b_sb["gamma"])
                nc.vector.tensor_mul(out=b_c, in0=mean_c, in1=a_c)
                nc.vector.tensor_tensor(out=b_c, in0=b_sb["beta"], in1=b_c,
                                        op=ALU.subtract)

                if dbg:
                    nc.sync.dma_start(out=dbg_y.rearrange("(k p) i -> p k i", p=128), in_=y_sb)
                    nc.sync.dma_start(out=dbg_st.rearrange("(k p) s -> p k s", p=128)[:, :, 0:1], in_=st4[:, :, 0:1])
                    nc.sync.dma_start(out=dbg_st.rearrange("(k p) s -> p k s", p=128)[:, :, 1:2], in_=st4[:, :, 1:2])
                yr = yh_d.rearrange("(k p) i -> p k i", p=128)
                for g in range(2):
                    for ci, (s, e) in enumerate(_mm_slices(NHALF)):
                        eng = nc.gpsimd if (ci == 1 and USE_GPSIMD) else nc.vector
                        eng.tensor_scalar(
                            out=y_sb[:, g, s:e], in0=y_sb[:, g, s:e],
                            scalar1=a_c[:, g : g + 1], scalar2=b_c[:, g : g + 1],
                            op0=ALU.mult, op1=ALU.add,
                        )
                        nc.sync.dma_start(out=yr[:, g, s:e], in_=y_sb[:, g, s:e])

    _finalize(nc)
    return nc


def _get_nc(dbg=False):
    key = ("ncd" if dbg else "nc")
    if key not in _CACHE:
        _CACHE[key] = _build(dbg)
    return _CACHE[key]


def make_in_maps(x, context, Wq, bq, Wk, bk, Wv, bv, Wo, bo, gamma, beta):
    x = np.asarray(x, np.float32)
    context = np.asarray(context, np.float32)
    xr = np.ascontiguousarray(x.reshape(B, C, HW).astype(np.float16))
    cr = np.ascontiguousarray(context.reshape(B, C, HW).astype(np.float16))

    gsel = np.zeros((128, 16), np.float32)
    gsel[np.arange(128), np.arange(128) // GSIZE] = 1.0 / GN_COUNT

    Wo32 = np.asarray(Wo, np.float32)
    bo_eff = np.asarray(bo, np.float32) + Wo32 @ np.asarray(bv, np.float32)

    shared = {
        "wqT": np.ascontiguousarray(np.asarray(Wq, np.float32).T.astype(np.float16)),
        "wkT": np.ascontiguousarray(np.asarray(Wk, np.float32).T.astype(np.float16)),
        "wvT": np.ascontiguousarray(np.asarray(Wv, np.float32).T.astype(np.float16)),
        "woT": np.ascontiguousarray(Wo32.T.astype(np.float16)),
        "bq": np.asarray(bq, np.float32).reshape(C, 1),
        "bk": np.asarray(bk, np.float32).reshape(C, 1),
        "bo": bo_eff.reshape(C, 1),
        "gamma": np.asarray(gamma, np.float32).reshape(C, 1),
        "beta": np.asarray(beta, np.float32).reshape(C, 1),
        "gsel": gsel,
        "gselT": np.ascontiguousarray((gsel.T != 0).astype(np.float32)),
    }
    in_maps = []
    for core in range(8):
        b, half = core // 2, core % 2
        m = dict(shared)
        m["xh"] = np.ascontiguousarray(xr[b, :, half * NHALF : (half + 1) * NHALF])
        m["ctx"] = cr[b]
        in_maps.append(m)
    return in_maps


def kernel(x, context, Wq, bq, Wk, bk, Wv, bv, Wo, bo, gamma, beta):
    in_maps = make_in_maps(
        x, context, Wq, bq, Wk, bk, Wv, bv, Wo, bo, gamma, beta
    )
    x = np.asarray(x, np.float32)

    nc = _get_nc()
    res = run_bass_kernel_spmd(nc, in_maps, core_ids=list(range(8)))

    out = np.empty((B, C, HW), np.float32)
    for core in range(8):
        b, half = core // 2, core % 2
        out[b, :, half * NHALF : (half + 1) * NHALF] = res.results[core]["yh"]
    return out.reshape(x.shape)


# revision 8
# speedup vs baseline: 1.0332x; 1.0332x over previous
"""Cross-attention + output projection + residual + GroupNorm on 8 NeuronCores.

Problem (hardcoded): B=4, C=256, H=W=48 (N=2304 pixels), 4 heads x 64 dim,
GroupNorm with 32 groups of 8 channels, eps=1e-5.

Sharding: 2 cores per batch element; each core handles one half of the
query pixels (1152) for all 4 heads.  K/V are computed for the full pixel
range on both cores of a pair (duplicated, cheap).  The only cross-core
communication is a 2KB AllReduce of per-channel (sum, sumsq) GroupNorm
partial statistics between the two cores of each pair.

v2 design (vs. the 169us baseline whose ACT engine was the 103us wall):
  * softmax exp is split across TWO engines, whole-tile alternating:
      ACT:  ex = Exp(scale * s)           (LUT, (N+352)/1.2 ns)
      DVE:  ex = bits_fp16(int16(A*s+B))  (Schraudolph 2^x bit trick,
            A = 1024*log2(e)*scale, B = 15*1024 - 45; ~3% rel err, which
            lands ~1e-3 on the final output -- validated vs the oracle)
  * score matmuls are row-tiled: each head's 64-dim contraction runs as
    two concurrent 64-row PE tiles (partitions 0-63 / 64-127) fed by
    partition-swapped duplicates q2/k2 (made by SBUF->SBUF DMA).
  * AV accumulates within the head's own slot (lag 2) so no AV work is
    left after the last exp; packed [i-tile x (64+ones)] PSUM banks, the
    bank-first matmul uses start=True instead of a pre-memset.
  * V projection runs up front in the projection-phase PSUM pool.
  * tail: Wo/residual/stats chunked right behind the last AV; channel
    (sum,sumsq) AllReduce launched as early as possible; final affine on
    DVE/gpsimd interleaved with the output DMA.
"""

import sys

if "/opt/trn_rl_repo" not in sys.path:
    sys.path.insert(0, "/opt/trn_rl_repo")

import math

import ml_dtypes
import numpy as np

import concourse.bass as bass
import concourse.mybir as mybir
import concourse.tile as tile
from concourse import bacc
from concourse.bass_utils import run_bass_kernel_spmd

F32 = mybir.dt.float32
BF16 = mybir.dt.bfloat16
F16 = mybir.dt.float16
I16 = mybir.dt.int16
AF = mybir.ActivationFunctionType
ALU = mybir.AluOpType
AXF = mybir.AxisListType.X

B, C, HW = 4, 256, 2304
NH, HD = 4, 64
NHALF = HW // 2  # 1152 query pixels per core
NJT = HW // 128  # 18 key tiles of 128
NIT = NHALF // 128  # 9 query tiles of 128
SCALE = HD ** -0.5  # 0.125
GSIZE = 8  # channels per GroupNorm group
EPS = 1e-5
GN_COUNT = GSIZE * HW  # elements per group per batch (after pair AllReduce)

# Schraudolph fp16 exp: exp(SCALE*s) ~= bits_fp16(int16(EXP_A*s + EXP_B))
EXP_A = 1024.0 * math.log2(math.e) * SCALE
EXP_B = 15.0 * 1024.0 - 45.0

NA = 5  # query tiles in AV accumulator bank A (rest in bank B)
AV_LAG = 2  # AV for (h, jt) issues in slot position jt+AV_LAG

import os as _os

def _flag(name, default):
    v = _os.environ.get(name)
    return default if v is None else v == "1"

LDW_REUSE = _flag("CA_LDW_REUSE", True)  # skip LDWEIGHTS when lhsT repeats
USE_GPSIMD = _flag("CA_GPSIMD", True)  # gpsimd for xbo staging / affine
USE_DVE_EXP = _flag("CA_DVE_EXP", True)  # Schraudolph exp tiles on DVE
AV_NO_MEMSET = _flag("CA_AV_NOMEMSET", True)  # bank-first start=True trick

# whole-tile exp engine assignment: number of DVE tiles per head
_DVE_PER_HEAD = (6, 8, 9, 9)


def _dve_exp_set():
    s = set()
    for h, n in enumerate(_DVE_PER_HEAD):
        for i in range(n):
            s.add((h, int((i + 0.5) * NJT / n)))
    return s


DVE_EXP = _dve_exp_set() if USE_DVE_EXP else set()

_CACHE = {}


def _mm_slices(total, step=512):
    return [(s, min(s + step, total)) for s in range(0, total, step)]


def _finalize(nc):
    """compile() leaves 3+-wait Matmults that walrus rejects ("Too many sync
    wait commands" on the S3_LW struct); a second compile pass — run here via
    finalize() — splits them onto EventSemaphores.  Verify that it worked."""
    nc.compile()
    nc.finalize()
    for fn in nc.m.functions:
        for bb in fn.blocks:
            for inst in bb.instructions:
                si = inst.sync_info
                if isinstance(inst, mybir.InstMatmult) and si is not None:
                    assert len(si.on_wait or []) <= 2, (inst.name, si.on_wait)


def _build(dbg=False):
    nc = bacc.Bacc("TRN2", target_bir_lowering=False, debug=False, num_devices=8)

    xh_d = nc.dram_tensor("xh", [C, NHALF], F16, kind="ExternalInput").ap()
    ctx_d = nc.dram_tensor("ctx", [C, HW], F16, kind="ExternalInput").ap()
    w_d = {
        nm: nc.dram_tensor(nm, [C, C], F16, kind="ExternalInput").ap()
        for nm in ("wqT", "wkT", "wvT", "woT")
    }
    b_d = {
        nm: nc.dram_tensor(nm, [C, 1], F32, kind="ExternalInput").ap()
        for nm in ("bq", "bk", "bo", "gamma", "beta")
    }
    gsel_d = nc.dram_tensor("gsel", [128, 16], F32, kind="ExternalInput").ap()
    gselT_d = nc.dram_tensor("gselT", [16, 128], F32, kind="ExternalInput").ap()
    yh_d = nc.dram_tensor("yh", [C, NHALF], F32, kind="ExternalOutput").ap()
    if dbg:
        dbg_q = nc.dram_tensor("dbg_q", [C, NHALF], F16, kind="ExternalOutput").ap()
        dbg_k = nc.dram_tensor("dbg_k", [C, HW], F16, kind="ExternalOutput").ap()
        dbg_q2 = nc.dram_tensor("dbg_q2", [C, NHALF], F16, kind="ExternalOutput").ap()
        dbg_ex = nc.dram_tensor("dbg_ex", [128, 2, NHALF], F16, kind="ExternalOutput").ap()
        dbg_v = nc.dram_tensor("dbg_v", [128, NJT, NH * (HD + 1)], F16, kind="ExternalOutput").ap()
        dbg_ao = nc.dram_tensor("dbg_ao", [C, NHALF], F16, kind="ExternalOutput").ap()
        dbg_y = nc.dram_tensor("dbg_y", [C, NHALF], F32, kind="ExternalOutput").ap()
        dbg_st = nc.dram_tensor("dbg_st", [C, 2], F32, kind="ExternalOutput").ap()
    warm_d = nc.dram_tensor("warm_out", [16, 1], F32, kind="ExternalOutput").ap()

    with tile.TileContext(nc) as tc:
        with (
            tc.tile_pool(name="const", bufs=1) as const,
            tc.tile_pool(name="main", bufs=1) as main,
            tc.tile_pool(name="small", bufs=4) as small,
            tc.tile_pool(name="dram", bufs=2, space="DRAM") as dram,
        ):
            # ---- constants (DMA order = consumption order) ----
            w_sb = {}
            for nm in ("wqT", "wkT", "wvT", "woT"):
                w_sb[nm] = const.tile([128, 2, C], F16, tag=nm, name=nm)
            b_sb = {}
            for nm in ("bq", "bk", "bo", "gamma", "beta"):
                b_sb[nm] = const.tile([128, 2], F32, tag=nm, name=nm)

            xh_sb = main.tile([128, 2, NHALF], F16, tag="xh")
            nc.sync.dma_start(
                out=w_sb["wqT"], in_=w_d["wqT"].rearrange("(k p) o -> p k o", p=128)
            )
            nc.sync.dma_start(out=xh_sb, in_=xh_d.rearrange("(k p) i -> p k i", p=128))
            nc.sync.dma_start(
                out=w_sb["wkT"], in_=w_d["wkT"].rearrange("(k p) o -> p k o", p=128)
            )

            # warm the Exp ACT table while input DMAs run
            eps_sb = const.tile([16, 1], F32, tag="eps")
            nc.vector.memset(eps_sb, EPS)
            warm_e = small.tile([16, 1], F32, tag="warme", bufs=1)
            nc.scalar.activation(out=warm_e, in_=eps_sb, func=AF.Exp)

            # q/k in head-parity layout: partitions 0-63 = head 2g,
            # 64-127 = head 2g+1 (natural channel order); q2/k2 are the
            # partition-swapped duplicates for row-tiled score matmuls.
            q_sb = main.tile([128, 2, NHALF], F16, tag="q")
            q2_sb = main.tile([128, 2, NHALF], F16, tag="q2")
            k_sb = main.tile([128, 2, HW], F16, tag="k")
            k2_sb = main.tile([128, 2, HW], F16, tag="k2")
            vT_sb = main.tile([128, NJT, NH * (HD + 1)], F16, tag="vT")
            ao_sb = main.tile([128, 2, NHALF], F16, tag="ao")
            aon_sb = main.tile([128, 2, NIT, 128], F16, tag="aon")
            xbo_sb = main.tile([128, 2, NHALF], F32, tag="xbo")
            y_sb = main.tile([128, 2, NHALF], F32, tag="y")
            scr_sb = main.tile([128, NHALF], F16, tag="scr")

            # ones columns of vT (one per head, strided over j-tiles)
            for h in range(NH):
                c0 = h * (HD + 1) + HD
                (nc.gpsimd if USE_GPSIMD else nc.vector).memset(vT_sb[:, :, c0 : c0 + 1], 1.0)

            ctx_sb = main.tile([128, 2, HW], F16, tag="ctx")

            # ---- projections (fp16) + V, all in the pp PSUM pool ----
            with (
                tc.tile_pool(name="pp", bufs=2, space="PSUM") as pp,
            ):
                ctx_r = ctx_d.rearrange("(k p) j -> p k j", p=128)
                for jq in range(4):
                    s0, s1 = jq * (HW // 4), (jq + 1) * (HW // 4)
                    nc.sync.dma_start(out=ctx_sb[:, :, s0:s1], in_=ctx_r[:, :, s0:s1])

                nc.sync.dma_start(
                    out=w_sb["wvT"], in_=w_d["wvT"].rearrange("(k p) o -> p k o", p=128)
                )
                nc.sync.dma_start(
                    out=w_sb["woT"], in_=w_d["woT"].rearrange("(k p) o -> p k o", p=128)
                )
                for nm in ("bq", "bk", "bo", "gamma", "beta"):
                    nc.sync.dma_start(
                        out=b_sb[nm],
                        in_=b_d[nm].rearrange("(k p) one -> p (k one)", p=128),
                    )
                gsel_sb = const.tile([128, 16], F32, tag="gsel")
                nc.sync.dma_start(out=gsel_sb, in_=gsel_d)
                gselT_sb = const.tile([16, 128], F32, tag="gselT")
                nc.sync.dma_start(out=gselT_sb, in_=gselT_d)

                # Q: [o_grp 128, 1152] -> q_sb, bias add fused in the copy
                for g in range(2):
                    ps = pp.tile([128, NHALF], F32, tag="qk")
                    for kk in range(2):
                        lhsT = w_sb["wqT"][:, kk, g * 128 : (g + 1) * 128]
                        for s, e in _mm_slices(NHALF):
                            nc.tensor.matmul(
                                ps[:, s:e], lhsT, xh_sb[:, kk, s:e],
                                start=(kk == 0), stop=(kk == 1),
                            )
                    nc.scalar.activation(
                        out=q_sb[:, g, :], in_=ps, func=AF.Identity,
                        bias=b_sb["bq"][:, g : g + 1],
                    )
                # duplicate with swapped partition halves (DMA, off-engine)
                nc.sync.dma_start(out=q2_sb[0:64, :, :], in_=q_sb[64:128, :, :])
                nc.sync.dma_start(out=q2_sb[64:128, :, :], in_=q_sb[0:64, :, :])

                # K: [o_grp 128, 2304] in two j-halves -> k_sb
                for jh in range(2):
                    for g in range(2):
                        ps = pp.tile([128, NHALF], F32, tag="qk")
                        for kk in range(2):
                            lhsT = w_sb["wkT"][:, kk, g * 128 : (g + 1) * 128]
                            for s, e in _mm_slices(NHALF):
                                nc.tensor.matmul(
                                    ps[:, s:e], lhsT,
                                    ctx_sb[:, kk, jh * NHALF + s : jh * NHALF + e],
                                    start=(kk == 0), stop=(kk == 1),
                                )
                        nc.scalar.activation(
                            out=k_sb[:, g, jh * NHALF : (jh + 1) * NHALF],
                            in_=ps, func=AF.Identity,
                            bias=b_sb["bk"][:, g : g + 1],
                        )
                    nc.sync.dma_start(
                        out=k2_sb[0:64, :, jh * NHALF : (jh + 1) * NHALF],
                        in_=k_sb[64:128, :, jh * NHALF : (jh + 1) * NHALF],
                    )
                    nc.sync.dma_start(
                        out=k2_sb[64:128, :, jh * NHALF : (jh + 1) * NHALF],
                        in_=k_sb[0:64, :, jh * NHALF : (jh + 1) * NHALF],
                    )

                # V: vT[j, c] = (ctx.T @ WvT)[j, c] per j-tile (up front)
                for jt in range(NJT):
                    vp = pp.tile([128, C], F32, tag="vp")
                    for kk in range(2):
                        nc.tensor.matmul(
                            vp, ctx_sb[:, kk, jt * 128 : (jt + 1) * 128],
                            w_sb["wvT"][:, kk, :],
                            start=(kk == 0), stop=(kk == 1),
                        )
                    nc.vector.tensor_copy(
                        out=vT_sb[:, jt, :]
                        .rearrange("p (h e) -> p h e", e=HD + 1)[:, :, :HD],
                        in_=vp.rearrange("p (h d) -> p h d", d=HD),
                    )

                # residual + bo staged in fp32 (gpsimd, off the hot engines)
                xbo_eng = nc.gpsimd if USE_GPSIMD else nc.vector
                for g in range(2):
                    xbo_eng.tensor_scalar_add(
                        out=xbo_sb[:, g, :], in0=xh_sb[:, g, :],
                        scalar1=b_sb["bo"][:, g : g + 1],
                    )

            # warm the collective path with a dummy tiny AllReduce overlapped
            # with attention -- absorbs first-use cc-stream/firmware latency
            # so the GN-tail AllReduce pays less.
            cwarm_sb = small.tile([16, 1], F32, tag="cwarm", bufs=1)
            nc.vector.memset(cwarm_sb, 1.0)
            cw_in = dram.tile([16, 1], F32, tag="cwin", bufs=1)
            cw_out = dram.tile([16, 1], F32, tag="cwout", bufs=1)
            nc.sync.dma_start(out=cw_in, in_=cwarm_sb)
            nc.gpsimd.collective_compute(
                "AllReduce", ALU.add,
                replica_groups=[[0, 1], [2, 3], [4, 5], [6, 7]],
                ins=[cw_in.opt()], outs=[cw_out.opt()],
            )

            # ---- attention: per head, AV trails scores/exp by AV_LAG ----
            with (
                tc.tile_pool(name="expp", bufs=6) as expp,
                tc.tile_pool(name="scp", bufs=2, space="PSUM") as scp,
                tc.tile_pool(name="avp", bufs=1, space="PSUM") as avp,
            ):
                ex_tiles = {}
                av_banks = {}
                rdens = {}

                def issue_scores(h, jt):
                    g, ph = h // 2, h % 2
                    qlo, klo = (q_sb, k_sb) if ph == 0 else (q2_sb, k2_sb)
                    qhi, khi = (q2_sb, k2_sb) if ph == 0 else (q_sb, k_sb)
                    sc = scp.tile([128, NHALF], F32, tag="sc", name=f"sc{h}_{jt}")
                    j0, j1 = jt * 128, (jt + 1) * 128
                    m1 = nc.tensor.matmul(
                        sc[:, 0:512], klo[0:64, g, j0:j1], qlo[0:64, g, 0:512],
                        start=True, stop=True,
                    )
                    nc.tensor.matmul(
                        sc[:, 512:1024], khi[64:128, g, j0:j1],
                        qhi[64:128, g, 512:1024],
                        start=True, stop=True,
                    )
                    m3 = nc.tensor.matmul(
                        sc[:, 1024:1152], klo[0:64, g, j0:j1],
                        qlo[0:64, g, 1024:1152],
                        start=True, stop=True,
                    )
                    if LDW_REUSE:
                        m3.ins.ldweights = False  # same lhsT as m1
                    return sc

                def issue_exp(h, jt, sc):
                    # split at the PSUM bank boundary: ACT reads bank 0,
                    # DVE banks 1-2 -- the two engines run concurrently.
                    ex = expp.tile([128, NHALF], F16, tag="exp", name=f"ex{h}_{jt}")
                    if USE_DVE_EXP:
                        nc.scalar.activation(out=ex[:, 0:512], in_=sc[:, 0:512],
                                             func=AF.Exp, scale=SCALE)
                        nc.vector.tensor_scalar(
                            out=ex[:, 512:NHALF].bitcast(I16),
                            in0=sc[:, 512:NHALF],
                            scalar1=EXP_A, scalar2=EXP_B,
                            op0=ALU.mult, op1=ALU.add,
                        )
                    else:
                        nc.scalar.activation(out=ex, in_=sc, func=AF.Exp,
                                             scale=SCALE)
                    ex_tiles[h, jt] = ex

                def issue_av(h, jt):
                    ava, avb = av_banks[h]
                    col = h * (HD + 1)
                    for it in range(NIT):
                        dst = (
                            ava[:, it * (HD + 1) : (it + 1) * (HD + 1)]
                            if it < NA
                            else avb[:, (it - NA) * (HD + 1) : (it - NA + 1) * (HD + 1)]
                        )
                        nc.tensor.matmul(
                            dst,
                            ex_tiles[h, jt][:, it * 128 : (it + 1) * 128],
                            vT_sb[:, jt, col : col + HD + 1],
                            start=(AV_NO_MEMSET and jt == 0 and it in (0, NA)),
                            stop=(jt == NJT - 1),
                            skip_group_check=True,
                        )
                    if jt == NJT - 1:
                        del ex_tiles[h, jt]  # allow pool rotation bookkeeping
                    else:
                        del ex_tiles[h, jt]

                def issue_norm(h, last):
                    """denominator reciprocal + normalize into aon."""
                    ava, avb = av_banks[h]
                    g, ph = h // 2, h % 2
                    rdena = small.tile([128, NA], F32, tag="rdena", bufs=2)
                    rdenb = small.tile([128, NIT - NA], F32, tag="rdenb", bufs=2)
                    nc.vector.reciprocal(out=rdena, in_=ava[:, HD :: HD + 1])
                    nc.vector.reciprocal(out=rdenb, in_=avb[:, HD :: HD + 1])
                    for it in range(NIT):
                        src = (
                            ava[:, it * (HD + 1) : it * (HD + 1) + HD]
                            if it < NA
                            else avb[:, (it - NA) * (HD + 1) : (it - NA) * (HD + 1) + HD]
                        )
                        rd = (
                            rdena[:, it : it + 1] if it < NA
                            else rdenb[:, it - NA : it - NA + 1]
                        )
                        out = aon_sb[:, g, it, ph * HD : (ph + 1) * HD]
                        nc.vector.tensor_scalar_mul(out=out, in0=src,
                                                    scalar1=rd)

                for h in range(NH):
                    av_banks[h] = (
                        avp.tile([128, NA * (HD + 1)], F32, tag="ava",
                                 name=f"ava{h}"),
                        avp.tile([128, (NIT - NA) * (HD + 1)], F32, tag="avb",
                                 name=f"avb{h}"),
                    )
                    if not AV_NO_MEMSET:
                        nc.vector.memset(av_banks[h][0], 0.0)
                        nc.vector.memset(av_banks[h][1], 0.0)
                    for jt in range(NJT):
                        sc = issue_scores(h, jt)
                        issue_exp(h, jt, sc)
                        if jt >= AV_LAG:
                            issue_av(h, jt - AV_LAG)
                    for jt in range(NJT - AV_LAG, NJT):
                        issue_av(h, jt)
                    issue_norm(h, last=(h == NH - 1))
                    if h % 2 == 1:  # both heads of the g-half staged
                        nc.sync.dma_start(
                            out=ao_sb[:, h // 2, :]
                            .rearrange("p (t i) -> p t i", i=128),
                            in_=aon_sb[:, h // 2, :, :],
                            transpose=True,
                        )

            if dbg:
                nc.sync.dma_start(out=dbg_q.rearrange("(k p) i -> p k i", p=128), in_=q_sb)
                nc.sync.dma_start(out=dbg_q2.rearrange("(k p) i -> p k i", p=128), in_=q2_sb)
                nc.sync.dma_start(out=dbg_k.rearrange("(k p) i -> p k i", p=128), in_=k_sb)
                nc.sync.dma_start(out=dbg_v, in_=vT_sb)
                nc.sync.dma_start(out=dbg_ao.rearrange("(k p) i -> p k i", p=128), in_=ao_sb)

            # Sqrt lives in a different ACT table set than Exp: warm the sqrt
            # table now (all exps done) so the GN-tail sqrt needs no load.
            # DMA to a real output so dead-code elimination keeps it.
            warm = small.tile([16, 1], F32, tag="warm", bufs=1)
            nc.scalar.sqrt(out=warm, in_=warm_e)
            nc.sync.dma_start(out=warm_d, in_=warm)

            # ---- output projection + residual + GroupNorm ----
            with (
                tc.tile_pool(name="wop", bufs=2, space="PSUM") as wop,
                tc.tile_pool(name="gnp", bufs=1, space="PSUM") as gnp,
            ):
                st4 = small.tile([128, 2, 2], F32, tag="st4", bufs=1)
                wps = []
                for kk in range(2):
                    for g in range(2):
                        if kk == 0:
                            wps.append(wop.tile([128, NHALF], F32, tag="wo",
                                                name=f"wo{g}"))
                        lhsT = w_sb["woT"][:, kk, g * 128 : (g + 1) * 128]
                        for s, e in _mm_slices(NHALF):
                            nc.tensor.matmul(wps[g][:, s:e], lhsT,
                                             ao_sb[:, kk, s:e],
                                             start=(kk == 0), stop=(kk == 1))
                for g in range(2):
                    nc.vector.tensor_tensor(
                        out=y_sb[:, g, :], in0=wps[g], in1=xbo_sb[:, g, :],
                        op=ALU.add,
                    )
                    nc.vector.reduce_sum(
                        out=st4[:, g, 0:1], in_=y_sb[:, g, :], axis=AXF
                    )
                    nc.scalar.activation(
                        out=scr_sb, in_=y_sb[:, g, :], func=AF.Square,
                        accum_out=st4[:, g, 1:2],
                    )

                # pair AllReduce of per-channel (sum, sumsq)
                gn_in = dram.tile([C, 2], F32, tag="gnin", bufs=1)
                gn_out = dram.tile([C, 2], F32, tag="gnout", bufs=1)
                nc.sync.dma_start(
                    out=gn_in.rearrange("(k p) s -> p k s", p=128), in_=st4
                )
                nc.gpsimd.collective_compute(
                    "AllReduce", ALU.add,
                    replica_groups=[[0, 1], [2, 3], [4, 5], [6, 7]],
                    ins=[gn_in.opt()], outs=[gn_out.opt()],
                )
                gs_sb = small.tile([128, 2, 2], F32, tag="gs", bufs=1)
                nc.sync.dma_start(
                    out=gs_sb, in_=gn_out.rearrange("(k p) s -> p k s", p=128)
                )

                # group totals via 0/1 selection matmul: [16 groups, (sum,sumsq)]
                gtot = small.tile([16, 2, 2], F32, tag="gtot", bufs=1)
                for kk in range(2):
                    gp = gnp.tile([16, 2], F32, tag="gp")
                    nc.tensor.matmul(gp, gsel_sb, gs_sb[:, kk, :],
                                     start=True, stop=True)
                    nc.vector.tensor_copy(out=gtot[:, kk, :], in_=gp)
                mean_g = gtot[:, :, 0]
                var_g = small.tile([16, 2], F32, tag="varg", bufs=1)
                nc.vector.tensor_copy(out=var_g, in_=gtot[:, :, 1])
                m2 = small.tile([16, 2], F32, tag="m2", bufs=1)
                nc.vector.tensor_mul(out=m2, in0=mean_g, in1=mean_g)
                nc.vector.tensor_tensor(out=var_g, in0=var_g, in1=m2,
                                        op=ALU.subtract)
                nc.scalar.activation(out=var_g, in_=var_g, func=AF.Sqrt,
                                     bias=eps_sb)
                nc.vector.reciprocal(out=var_g, in_=var_g)  # rstd [16, 2]

                # broadcast group stats to channels, fold gamma/beta into A,B
                mean_c = small.tile([128, 2], F32, tag="meanc", bufs=1)
                rstd_c = small.tile([128, 2], F32, tag="rstdc", bufs=1)
                for src, dst in ((mean_g, mean_c), (var_g, rstd_c)):
                    gp = gnp.tile([128, 2], F32, tag="gb")
                    nc.tensor.matmul(gp, gselT_sb, src, start=True, stop=True)
                    nc.vector.tensor_copy(out=dst, in_=gp)
                a_c = small.tile([128, 2], F32, tag="ac", bufs=1)
                b_c = small.tile([128, 2], F32, tag="bc", bufs=1)
                nc.vector.tensor_mul(out=a_c, in0=rstd_c, in1=b_sb["gamma"])
                nc.vector.tensor_mul(out=b_c, in0=mean_c, in1=a_c)
                nc.vector.tensor_tensor(out=b_c, in0=b_sb["beta"], in1=b_c,
                                        op=ALU.subtract)

                if dbg:
                    nc.sync.dma_start(out=dbg_y.rearrange("(k p) i -> p k i", p=128), in_=y_sb)
                    nc.sync.dma_start(out=dbg_st.rearrange("(k p) s -> p k s", p=128)[:, :, 0:1], in_=st4[:, :, 0:1])
                    nc.sync.dma_start(out=dbg_st.rearrange("(k p) s -> p k s", p=128)[:, :, 1:2], in_=st4[:, :, 1:2])
                yr = yh_d.rearrange("(k p) i -> p k i", p=128)
                for g in range(2):
                    for ci, (s, e) in enumerate(_mm_slices(NHALF)):
                        eng = nc.gpsimd if (ci == 1 and USE_GPSIMD) else nc.vector
                        eng.tensor_scalar(
                            out=y_sb[:, g, s:e], in0=y_sb[:, g, s:e],
                            scalar1=a_c[:, g : g + 1], scalar2=b_c[:, g : g + 1],
                            op0=ALU.mult, op1=ALU.add,
                        )
                        nc.sync.dma_start(out=yr[:, g, s:e], in_=y_sb[:, g, s:e])

    _finalize(nc)
    return nc


def _get_nc(dbg=False):
    key = ("ncd" if dbg else "nc")
    if key not in _CACHE:
        _CACHE[key] = _build(dbg)
    return _CACHE[key]


def make_in_maps(x, context, Wq, bq, Wk, bk, Wv, bv, Wo, bo, gamma, beta):
    x = np.asarray(x, np.float32)
    context = np.asarray(context, np.float32)
    xr = np.ascontiguousarray(x.reshape(B, C, HW).astype(np.float16))
    cr = np.ascontiguousarray(context.reshape(B, C, HW).astype(np.float16))

    gsel = np.zeros((128, 16), np.float32)
    gsel[np.arange(128), np.arange(128) // GSIZE] = 1.0 / GN_COUNT

    Wo32 = np.asarray(Wo, np.float32)
    bo_eff = np.asarray(bo, np.float32) + Wo32 @ np.asarray(bv, np.float32)

    shared = {
        "wqT": np.ascontiguousarray(np.asarray(Wq, np.float32).T.astype(np.float16)),
        "wkT": np.ascontiguousarray(np.asarray(Wk, np.float32).T.astype(np.float16)),
        "wvT": np.ascontiguousarray(np.asarray(Wv, np.float32).T.astype(np.float16)),
        "woT": np.ascontiguousarray(Wo32.T.astype(np.float16)),
        "bq": np.asarray(bq, np.float32).reshape(C, 1),
        "bk": np.asarray(bk, np.float32).reshape(C, 1),
        "bo": bo_eff.reshape(C, 1),
        "gamma": np.asarray(gamma, np.float32).reshape(C, 1),
        "beta": np.asarray(beta, np.float32).reshape(C, 1),
        "gsel": gsel,
        "gselT": np.ascontiguousarray((gsel.T != 0).astype(np.float32)),
    }
    in_maps = []
    for core in range(8):
        b, half = core // 2, core % 2
        m = dict(shared)
        m["xh"] = np.ascontiguousarray(xr[b, :, half * NHALF : (half + 1) * NHALF])
        m["ctx"] = cr[b]
        in_maps.append(m)
    return in_maps


def kernel(x, context, Wq, bq, Wk, bk, Wv, bv, Wo, bo, gamma, beta):
    in_maps = make_in_maps(
        x, context, Wq, bq, Wk, bk, Wv, bv, Wo, bo, gamma, beta
    )
    x = np.asarray(x, np.float32)

    nc = _get_nc()
    res = run_bass_kernel_spmd(nc, in_maps, core_ids=list(range(8)))

    out = np.empty((B, C, HW), np.float32)
    for core in range(8):
        b, half = core // 2, core % 2
        out[b, :, half * NHALF : (half + 1) * NHALF] = res.results[core]["yh"]
    return out.reshape(x.shape)


# revision 9
# speedup vs baseline: 1.1317x; 1.0954x over previous
"""Cross-attention + output projection + residual + GroupNorm on 8 NeuronCores.

Problem (hardcoded): B=4, C=256, H=W=48 (N=2304 pixels), 4 heads x 64 dim,
GroupNorm with 32 groups of 8 channels, eps=1e-5.

Sharding: 2 cores per batch element; each core handles one half of the
query pixels (1152) for all 4 heads.  K/V are computed for the full pixel
range on both cores of a pair (duplicated, cheap).  The only cross-core
communication is a 2KB AllReduce of per-channel (sum, sumsq) GroupNorm
partial statistics between the two cores of each pair.

v2 design (vs. the 169us baseline whose ACT engine was the 103us wall):
  * softmax exp is split across TWO engines, whole-tile alternating:
      ACT:  ex = Exp(scale * s)           (LUT, (N+352)/1.2 ns)
      DVE:  ex = bits_fp16(int16(A*s+B))  (Schraudolph 2^x bit trick,
            A = 1024*log2(e)*scale, B = 15*1024 - 45; ~3% rel err, which
            lands ~1e-3 on the final output -- validated vs the oracle)
  * score matmuls are row-tiled: each head's 64-dim contraction runs as
    two concurrent 64-row PE tiles (partitions 0-63 / 64-127) fed by
    partition-swapped duplicates q2/k2 (made by SBUF->SBUF DMA).
  * AV accumulates within the head's own slot (lag 2) so no AV work is
    left after the last exp; packed [i-tile x (64+ones)] PSUM banks, the
    bank-first matmul uses start=True instead of a pre-memset.
  * V projection runs up front in the projection-phase PSUM pool.
  * tail: Wo/residual/stats chunked right behind the last AV; channel
    (sum,sumsq) AllReduce launched as early as possible; final affine on
    DVE/gpsimd interleaved with the output DMA.
"""

import sys

if "/opt/trn_rl_repo" not in sys.path:
    sys.path.insert(0, "/opt/trn_rl_repo")

import math

import ml_dtypes
import numpy as np

import concourse.bass as bass
import concourse.mybir as mybir
import concourse.tile as tile
from concourse import bacc
from concourse.bass_utils import run_bass_kernel_spmd

F32 = mybir.dt.float32
BF16 = mybir.dt.bfloat16
F16 = mybir.dt.float16
I16 = mybir.dt.int16
AF = mybir.ActivationFunctionType
ALU = mybir.AluOpType
AXF = mybir.AxisListType.X

B, C, HW = 4, 256, 2304
NH, HD = 4, 64
NHALF = HW // 2  # 1152 query pixels per core
NJT = HW // 128  # 18 key tiles of 128
NIT = NHALF // 128  # 9 query tiles of 128
SCALE = HD ** -0.5  # 0.125
GSIZE = 8  # channels per GroupNorm group
EPS = 1e-5
GN_COUNT = GSIZE * HW  # elements per group per batch (after pair AllReduce)

# Schraudolph fp16 exp: exp(SCALE*s) ~= bits_fp16(int16(EXP_A*s + EXP_B))
EXP_A = 1024.0 * math.log2(math.e) * SCALE
EXP_B = 15.0 * 1024.0 - 45.0

NA = 5  # query tiles in AV accumulator bank A (rest in bank B)
AV_LAG = 2  # AV for (h, jt) issues in slot position jt+AV_LAG

import os as _os

def _flag(name, default):
    v = _os.environ.get(name)
    return default if v is None else v == "1"

LDW_REUSE = _flag("CA_LDW_REUSE", True)  # skip LDWEIGHTS when lhsT repeats
USE_GPSIMD = _flag("CA_GPSIMD", True)  # gpsimd for xbo staging / affine
USE_DVE_EXP = _flag("CA_DVE_EXP", True)  # Schraudolph exp tiles on DVE
AV_NO_MEMSET = _flag("CA_AV_NOMEMSET", True)  # bank-first start=True trick

# whole-tile exp engine assignment: number of DVE tiles per head
_DVE_PER_HEAD = (6, 8, 9, 9)


def _dve_exp_set():
    s = set()
    for h, n in enumerate(_DVE_PER_HEAD):
        for i in range(n):
            s.add((h, int((i + 0.5) * NJT / n)))
    return s


DVE_EXP = _dve_exp_set() if USE_DVE_EXP else set()

_CACHE = {}


def _mm_slices(total, step=512):
    return [(s, min(s + step, total)) for s in range(0, total, step)]


def _finalize(nc):
    """compile() leaves 3+-wait Matmults that walrus rejects ("Too many sync
    wait commands" on the S3_LW struct); a second compile pass — run here via
    finalize() — splits them onto EventSemaphores.  Verify that it worked."""
    nc.compile()
    nc.finalize()
    for fn in nc.m.functions:
        for bb in fn.blocks:
            for inst in bb.instructions:
                si = inst.sync_info
                if isinstance(inst, mybir.InstMatmult) and si is not None:
                    assert len(si.on_wait or []) <= 2, (inst.name, si.on_wait)


def _build(dbg=False):
    nc = bacc.Bacc("TRN2", target_bir_lowering=False, debug=False, num_devices=8)

    xh_d = nc.dram_tensor("xh", [C, NHALF], F16, kind="ExternalInput").ap()
    ctx_d = nc.dram_tensor("ctx", [C, HW], F16, kind="ExternalInput").ap()
    w_d = {
        nm: nc.dram_tensor(nm, [C, C], F16, kind="ExternalInput").ap()
        for nm in ("wqT", "wkT", "wvT", "woT")
    }
    b_d = {
        nm: nc.dram_tensor(nm, [C, 1], F32, kind="ExternalInput").ap()
        for nm in ("bq", "bk", "bo", "gamma", "beta")
    }
    gsel_d = nc.dram_tensor("gsel", [128, 16], F32, kind="ExternalInput").ap()
    gselT_d = nc.dram_tensor("gselT", [16, 128], F32, kind="ExternalInput").ap()
    yh_d = nc.dram_tensor("yh", [C, NHALF], F32, kind="ExternalOutput").ap()
    if dbg:
        dbg_q = nc.dram_tensor("dbg_q", [C, NHALF], F16, kind="ExternalOutput").ap()
        dbg_k = nc.dram_tensor("dbg_k", [C, HW], F16, kind="ExternalOutput").ap()
        dbg_q2 = nc.dram_tensor("dbg_q2", [C, NHALF], F16, kind="ExternalOutput").ap()
        dbg_ex = nc.dram_tensor("dbg_ex", [128, 2, NHALF], F16, kind="ExternalOutput").ap()
        dbg_v = nc.dram_tensor("dbg_v", [128, NJT, NH * (HD + 1)], F16, kind="ExternalOutput").ap()
        dbg_ao = nc.dram_tensor("dbg_ao", [C, NHALF], F16, kind="ExternalOutput").ap()
        dbg_y = nc.dram_tensor("dbg_y", [C, NHALF], F32, kind="ExternalOutput").ap()
        dbg_st = nc.dram_tensor("dbg_st", [C, 2], F32, kind="ExternalOutput").ap()
    warm_d = nc.dram_tensor("warm_out", [16, 1], F32, kind="ExternalOutput").ap()

    with tile.TileContext(nc) as tc:
        with (
            tc.tile_pool(name="const", bufs=1) as const,
            tc.tile_pool(name="main", bufs=1) as main,
            tc.tile_pool(name="small", bufs=4) as small,
            tc.tile_pool(name="dram", bufs=2, space="DRAM") as dram,
        ):
            # ---- constants (DMA order = consumption order) ----
            w_sb = {}
            for nm in ("wqT", "wkT", "wvT", "woT"):
                w_sb[nm] = const.tile([128, 2, C], F16, tag=nm, name=nm)
            b_sb = {}
            for nm in ("bq", "bk", "bo", "gamma", "beta"):
                b_sb[nm] = const.tile([128, 2], F32, tag=nm, name=nm)

            xh_sb = main.tile([128, 2, NHALF], F16, tag="xh")
            nc.sync.dma_start(
                out=w_sb["wqT"], in_=w_d["wqT"].rearrange("(k p) o -> p k o", p=128)
            )
            nc.sync.dma_start(out=xh_sb, in_=xh_d.rearrange("(k p) i -> p k i", p=128))
            nc.sync.dma_start(
                out=w_sb["wkT"], in_=w_d["wkT"].rearrange("(k p) o -> p k o", p=128)
            )

            # warm the Exp ACT table while input DMAs run
            eps_sb = const.tile([16, 1], F32, tag="eps")
            nc.vector.memset(eps_sb, EPS)
            warm_e = small.tile([16, 1], F32, tag="warme", bufs=1)
            nc.scalar.activation(out=warm_e, in_=eps_sb, func=AF.Exp)

            # q/k in head-parity layout: partitions 0-63 = head 2g,
            # 64-127 = head 2g+1 (natural channel order); q2/k2 are the
            # partition-swapped duplicates for row-tiled score matmuls.
            q_sb = main.tile([128, 2, NHALF], F16, tag="q")
            q2_sb = main.tile([128, 2, NHALF], F16, tag="q2")
            k_sb = main.tile([128, 2, HW], F16, tag="k")
            k2_sb = main.tile([128, 2, HW], F16, tag="k2")
            vT_sb = main.tile([128, NJT, NH * (HD + 1)], F16, tag="vT")
            ao_sb = main.tile([128, 2, NHALF], F16, tag="ao")
            aon_sb = main.tile([128, 2, NIT, 128], F16, tag="aon")
            xbo_sb = main.tile([128, 2, NHALF], F32, tag="xbo")
            y_sb = main.tile([128, 2, NHALF], F32, tag="y")
            scr_sb = main.tile([128, NHALF], F16, tag="scr")

            # ones columns of vT (one per head, strided over j-tiles)
            for h in range(NH):
                c0 = h * (HD + 1) + HD
                (nc.gpsimd if USE_GPSIMD else nc.vector).memset(vT_sb[:, :, c0 : c0 + 1], 1.0)

            ctx_sb = main.tile([128, 2, HW], F16, tag="ctx")

            # ---- projections (fp16) + V, all in the pp PSUM pool ----
            with (
                tc.tile_pool(name="pp", bufs=2, space="PSUM") as pp,
            ):
                ctx_r = ctx_d.rearrange("(k p) j -> p k j", p=128)
                for jq in range(4):
                    s0, s1 = jq * (HW // 4), (jq + 1) * (HW // 4)
                    nc.sync.dma_start(out=ctx_sb[:, :, s0:s1], in_=ctx_r[:, :, s0:s1])

                nc.sync.dma_start(
                    out=w_sb["wvT"], in_=w_d["wvT"].rearrange("(k p) o -> p k o", p=128)
                )
                nc.sync.dma_start(
                    out=w_sb["woT"], in_=w_d["woT"].rearrange("(k p) o -> p k o", p=128)
                )
                for nm in ("bq", "bk", "bo", "gamma", "beta"):
                    nc.sync.dma_start(
                        out=b_sb[nm],
                        in_=b_d[nm].rearrange("(k p) one -> p (k one)", p=128),
                    )
                gsel_sb = const.tile([128, 16], F32, tag="gsel")
                nc.sync.dma_start(out=gsel_sb, in_=gsel_d)
                gselT_sb = const.tile([16, 128], F32, tag="gselT")
                nc.sync.dma_start(out=gselT_sb, in_=gselT_d)

                # Q: [o_grp 128, 1152] -> q_sb, bias add fused in the copy
                for g in range(2):
                    ps = pp.tile([128, NHALF], F32, tag="qk")
                    for kk in range(2):
                        lhsT = w_sb["wqT"][:, kk, g * 128 : (g + 1) * 128]
                        for s, e in _mm_slices(NHALF):
                            nc.tensor.matmul(
                                ps[:, s:e], lhsT, xh_sb[:, kk, s:e],
                                start=(kk == 0), stop=(kk == 1),
                            )
                    nc.scalar.activation(
                        out=q_sb[:, g, :], in_=ps, func=AF.Identity,
                        bias=b_sb["bq"][:, g : g + 1],
                    )
                # duplicate with swapped partition halves (DMA, off-engine)
                nc.sync.dma_start(out=q2_sb[0:64, :, :], in_=q_sb[64:128, :, :])
                nc.sync.dma_start(out=q2_sb[64:128, :, :], in_=q_sb[0:64, :, :])

                # K: [o_grp 128, 2304] in two j-halves -> k_sb
                for jh in range(2):
                    for g in range(2):
                        ps = pp.tile([128, NHALF], F32, tag="qk")
                        for kk in range(2):
                            lhsT = w_sb["wkT"][:, kk, g * 128 : (g + 1) * 128]
                            for s, e in _mm_slices(NHALF):
                                nc.tensor.matmul(
                                    ps[:, s:e], lhsT,
                                    ctx_sb[:, kk, jh * NHALF + s : jh * NHALF + e],
                                    start=(kk == 0), stop=(kk == 1),
                                )
                        nc.scalar.activation(
                            out=k_sb[:, g, jh * NHALF : (jh + 1) * NHALF],
                            in_=ps, func=AF.Identity,
                            bias=b_sb["bk"][:, g : g + 1],
                        )
                    nc.sync.dma_start(
                        out=k2_sb[0:64, :, jh * NHALF : (jh + 1) * NHALF],
                        in_=k_sb[64:128, :, jh * NHALF : (jh + 1) * NHALF],
                    )
                    nc.sync.dma_start(
                        out=k2_sb[64:128, :, jh * NHALF : (jh + 1) * NHALF],
                        in_=k_sb[0:64, :, jh * NHALF : (jh + 1) * NHALF],
                    )

                # V: vT[j, c] = (ctx.T @ WvT)[j, c] per j-tile (up front)
                for jt in range(NJT):
                    vp = pp.tile([128, C], F32, tag="vp")
                    for kk in range(2):
                        nc.tensor.matmul(
                            vp, ctx_sb[:, kk, jt * 128 : (jt + 1) * 128],
                            w_sb["wvT"][:, kk, :],
                            start=(kk == 0), stop=(kk == 1),
                        )
                    nc.vector.tensor_copy(
                        out=vT_sb[:, jt, :]
                        .rearrange("p (h e) -> p h e", e=HD + 1)[:, :, :HD],
                        in_=vp.rearrange("p (h d) -> p h d", d=HD),
                    )

                # residual + bo staged in fp32 (gpsimd, off the hot engines)
                xbo_eng = nc.gpsimd if USE_GPSIMD else nc.vector
                for g in range(2):
                    xbo_eng.tensor_scalar_add(
                        out=xbo_sb[:, g, :], in0=xh_sb[:, g, :],
                        scalar1=b_sb["bo"][:, g : g + 1],
                    )

            # warm the collective path with a dummy tiny AllReduce overlapped
            # with attention -- absorbs first-use cc-stream/firmware latency
            # so the GN-tail AllReduce pays less.
            cwarm_sb = small.tile([16, 1], F32, tag="cwarm", bufs=1)
            nc.vector.memset(cwarm_sb, 1.0)
            cw_in = dram.tile([16, 1], F32, tag="cwin", bufs=1)
            cw_out = dram.tile([16, 1], F32, tag="cwout", bufs=1)
            nc.sync.dma_start(out=cw_in, in_=cwarm_sb)
            nc.gpsimd.collective_compute(
                "AllReduce", ALU.add,
                replica_groups=[[0, 1], [2, 3], [4, 5], [6, 7]],
                ins=[cw_in.opt()], outs=[cw_out.opt()],
            )

            # ---- attention: per head, AV trails scores/exp by AV_LAG ----
            with (
                tc.tile_pool(name="expp", bufs=6) as expp,
                tc.tile_pool(name="scp", bufs=2, space="PSUM") as scp,
                tc.tile_pool(name="avp", bufs=1, space="PSUM") as avp,
            ):
                ex_tiles = {}
                av_banks = {}
                rdens = {}

                def issue_scores(h, jt):
                    # two PSUM tiles per slot so the ACT / DVE exp halves
                    # release their buffers independently:
                    #   sc_a [128,512] (1 bank):  q[0:512]    on rows 0-63
                    #   sc_b [128,640] (2 banks): q[512:1024] on rows 64-127
                    #                             q[1024:1152] on rows 0-63
                    g, ph = h // 2, h % 2
                    qlo, klo = (q_sb, k_sb) if ph == 0 else (q2_sb, k2_sb)
                    qhi, khi = (q2_sb, k2_sb) if ph == 0 else (q_sb, k_sb)
                    sca = scp.tile([128, 512], F32, tag="sca", name=f"sca{h}_{jt}")
                    scb = scp.tile([128, 640], F32, tag="scb", name=f"scb{h}_{jt}")
                    j0, j1 = jt * 128, (jt + 1) * 128
                    nc.tensor.matmul(
                        scb[:, 512:640], klo[0:64, g, j0:j1],
                        qlo[0:64, g, 1024:1152],
                        start=True, stop=True,
                    )
                    nc.tensor.matmul(
                        scb[:, 0:512], khi[64:128, g, j0:j1],
                        qhi[64:128, g, 512:1024],
                        start=True, stop=True,
                    )
                    m3 = nc.tensor.matmul(
                        sca[:, 0:512], klo[0:64, g, j0:j1],
                        qlo[0:64, g, 0:512],
                        start=True, stop=True,
                    )
                    if LDW_REUSE:
                        m3.ins.ldweights = False  # same lhsT as the first MM
                    return sca, scb

                def issue_exp(h, jt, sc):
                    # ACT handles q[0:512] (sc_a), DVE q[512:1152] (sc_b) --
                    # different PSUM banks, so the engines run concurrently.
                    sca, scb = sc
                    ex = expp.tile([128, NHALF], F16, tag="exp", name=f"ex{h}_{jt}")
                    if USE_DVE_EXP:
                        nc.scalar.activation(out=ex[:, 0:512], in_=sca,
                                             func=AF.Exp, scale=SCALE)
                        nc.vector.tensor_scalar(
                            out=ex[:, 512:NHALF].bitcast(I16),
                            in0=scb,
                            scalar1=EXP_A, scalar2=EXP_B,
                            op0=ALU.mult, op1=ALU.add,
                        )
                    else:
                        nc.scalar.activation(out=ex[:, 0:512], in_=sca,
                                             func=AF.Exp, scale=SCALE)
                        nc.scalar.activation(out=ex[:, 512:NHALF], in_=scb,
                                             func=AF.Exp, scale=SCALE)
                    ex_tiles[h, jt] = ex

                def issue_av(h, jt):
                    ava, avb = av_banks[h]
                    col = h * (HD + 1)
                    for it in range(NIT):
                        dst = (
                            ava[:, it * (HD + 1) : (it + 1) * (HD + 1)]
                            if it < NA
                            else avb[:, (it - NA) * (HD + 1) : (it - NA + 1) * (HD + 1)]
                        )
                        nc.tensor.matmul(
                            dst,
                            ex_tiles[h, jt][:, it * 128 : (it + 1) * 128],
                            vT_sb[:, jt, col : col + HD + 1],
                            start=(AV_NO_MEMSET and jt == 0 and it in (0, NA)),
                            stop=(jt == NJT - 1),
                            skip_group_check=True,
                        )
                    if jt == NJT - 1:
                        del ex_tiles[h, jt]  # allow pool rotation bookkeeping
                    else:
                        del ex_tiles[h, jt]

                def issue_norm(h, last):
                    """denominator reciprocal + normalize into aon."""
                    ava, avb = av_banks[h]
                    g, ph = h // 2, h % 2
                    rdena = small.tile([128, NA], F32, tag="rdena", bufs=2)
                    rdenb = small.tile([128, NIT - NA], F32, tag="rdenb", bufs=2)
                    nc.vector.reciprocal(out=rdena, in_=ava[:, HD :: HD + 1])
                    nc.vector.reciprocal(out=rdenb, in_=avb[:, HD :: HD + 1])
                    for it in range(NIT):
                        src = (
                            ava[:, it * (HD + 1) : it * (HD + 1) + HD]
                            if it < NA
                            else avb[:, (it - NA) * (HD + 1) : (it - NA) * (HD + 1) + HD]
                        )
                        rd = (
                            rdena[:, it : it + 1] if it < NA
                            else rdenb[:, it - NA : it - NA + 1]
                        )
                        out = aon_sb[:, g, it, ph * HD : (ph + 1) * HD]
                        nc.vector.tensor_scalar_mul(out=out, in0=src,
                                                    scalar1=rd)

                for h in range(NH):
                    av_banks[h] = (
                        avp.tile([128, NA * (HD + 1)], F32, tag="ava",
                                 name=f"ava{h}"),
                        avp.tile([128, (NIT - NA) * (HD + 1)], F32, tag="avb",
                                 name=f"avb{h}"),
                    )
                    if not AV_NO_MEMSET:
                        nc.vector.memset(av_banks[h][0], 0.0)
                        nc.vector.memset(av_banks[h][1], 0.0)
                    for jt in range(NJT):
                        sc = issue_scores(h, jt)
                        issue_exp(h, jt, sc)
                        if jt >= AV_LAG:
                            issue_av(h, jt - AV_LAG)
                    for jt in range(NJT - AV_LAG, NJT):
                        issue_av(h, jt)
                    issue_norm(h, last=(h == NH - 1))
                    if h % 2 == 1:  # both heads of the g-half staged
                        nc.sync.dma_start(
                            out=ao_sb[:, h // 2, :]
                            .rearrange("p (t i) -> p t i", i=128),
                            in_=aon_sb[:, h // 2, :, :],
                            transpose=True,
                        )

            if dbg:
                nc.sync.dma_start(out=dbg_q.rearrange("(k p) i -> p k i", p=128), in_=q_sb)
                nc.sync.dma_start(out=dbg_q2.rearrange("(k p) i -> p k i", p=128), in_=q2_sb)
                nc.sync.dma_start(out=dbg_k.rearrange("(k p) i -> p k i", p=128), in_=k_sb)
                nc.sync.dma_start(out=dbg_v, in_=vT_sb)
                nc.sync.dma_start(out=dbg_ao.rearrange("(k p) i -> p k i", p=128), in_=ao_sb)

            # Sqrt lives in a different ACT table set than Exp: warm the sqrt
            # table now (all exps done) so the GN-tail sqrt needs no load.
            # DMA to a real output so dead-code elimination keeps it.
            warm = small.tile([16, 1], F32, tag="warm", bufs=1)
            nc.scalar.sqrt(out=warm, in_=warm_e)
            nc.sync.dma_start(out=warm_d, in_=warm)

            # ---- output projection + residual + GroupNorm ----
            with (
                tc.tile_pool(name="wop", bufs=2, space="PSUM") as wop,
                tc.tile_pool(name="gnp", bufs=1, space="PSUM") as gnp,
            ):
                st4 = small.tile([128, 2, 2], F32, tag="st4", bufs=1)
                wps = []
                for kk in range(2):
                    for g in range(2):
                        if kk == 0:
                            wps.append(wop.tile([128, NHALF], F32, tag="wo",
                                                name=f"wo{g}"))
                        lhsT = w_sb["woT"][:, kk, g * 128 : (g + 1) * 128]
                        for s, e in _mm_slices(NHALF):
                            nc.tensor.matmul(wps[g][:, s:e], lhsT,
                                             ao_sb[:, kk, s:e],
                                             start=(kk == 0), stop=(kk == 1))
                for g in range(2):
                    nc.vector.tensor_tensor(
                        out=y_sb[:, g, :], in0=wps[g], in1=xbo_sb[:, g, :],
                        op=ALU.add,
                    )
                    nc.vector.reduce_sum(
                        out=st4[:, g, 0:1], in_=y_sb[:, g, :], axis=AXF
                    )
                    nc.scalar.activation(
                        out=scr_sb, in_=y_sb[:, g, :], func=AF.Square,
                        accum_out=st4[:, g, 1:2],
                    )

                # pair AllReduce of per-channel (sum, sumsq)
                gn_in = dram.tile([C, 2], F32, tag="gnin", bufs=1)
                gn_out = dram.tile([C, 2], F32, tag="gnout", bufs=1)
                nc.sync.dma_start(
                    out=gn_in.rearrange("(k p) s -> p k s", p=128), in_=st4
                )
                nc.gpsimd.collective_compute(
                    "AllReduce", ALU.add,
                    replica_groups=[[0, 1], [2, 3], [4, 5], [6, 7]],
                    ins=[gn_in.opt()], outs=[gn_out.opt()],
                )
                gs_sb = small.tile([128, 2, 2], F32, tag="gs", bufs=1)
                nc.sync.dma_start(
                    out=gs_sb, in_=gn_out.rearrange("(k p) s -> p k s", p=128)
                )

                # group totals via 0/1 selection matmul: [16 groups, (sum,sumsq)]
                gtot = small.tile([16, 2, 2], F32, tag="gtot", bufs=1)
                for kk in range(2):
                    gp = gnp.tile([16, 2], F32, tag="gp")
                    nc.tensor.matmul(gp, gsel_sb, gs_sb[:, kk, :],
                                     start=True, stop=True)
                    nc.vector.tensor_copy(out=gtot[:, kk, :], in_=gp)
                mean_g = gtot[:, :, 0]
                var_g = small.tile([16, 2], F32, tag="varg", bufs=1)
                nc.vector.tensor_copy(out=var_g, in_=gtot[:, :, 1])
                m2 = small.tile([16, 2], F32, tag="m2", bufs=1)
                nc.vector.tensor_mul(out=m2, in0=mean_g, in1=mean_g)
                nc.vector.tensor_tensor(out=var_g, in0=var_g, in1=m2,
                                        op=ALU.subtract)
                nc.scalar.activation(out=var_g, in_=var_g, func=AF.Sqrt,
                                     bias=eps_sb)
                nc.vector.reciprocal(out=var_g, in_=var_g)  # rstd [16, 2]

                # broadcast group stats to channels, fold gamma/beta into A,B
                mean_c = small.tile([128, 2], F32, tag="meanc", bufs=1)
                rstd_c = small.tile([128, 2], F32, tag="rstdc", bufs=1)
                for src, dst in ((mean_g, mean_c), (var_g, rstd_c)):
                    gp = gnp.tile([128, 2], F32, tag="gb")
                    nc.tensor.matmul(gp, gselT_sb, src, start=True, stop=True)
                    nc.vector.tensor_copy(out=dst, in_=gp)
                a_c = small.tile([128, 2], F32, tag="ac", bufs=1)
                b_c = small.tile([128, 2], F32, tag="bc", bufs=1)
                nc.vector.tensor_mul(out=a_c, in0=rstd_c, in1=b_sb["gamma"])
                nc.vector.tensor_mul(out=b_c, in0=mean_c, in1=a_c)
                nc.vector.tensor_tensor(out=b_c, in0=b_sb["beta"], in1=b_c,
                                        op=ALU.subtract)

                if dbg:
                    nc.sync.dma_start(out=dbg_y.rearrange("(k p) i -> p k i", p=128), in_=y_sb)
                    nc.sync.dma_start(out=dbg_st.rearrange("(k p) s -> p k s", p=128)[:, :, 0:1], in_=st4[:, :, 0:1])
                    nc.sync.dma_start(out=dbg_st.rearrange("(k p) s -> p k s", p=128)[:, :, 1:2], in_=st4[:, :, 1:2])
                yr = yh_d.rearrange("(k p) i -> p k i", p=128)
                for g in range(2):
                    for ci, (s, e) in enumerate(_mm_slices(NHALF)):
                        eng = nc.gpsimd if (ci == 1 and USE_GPSIMD) else nc.vector
                        eng.tensor_scalar(
                            out=y_sb[:, g, s:e], in0=y_sb[:, g, s:e],
                            scalar1=a_c[:, g : g + 1], scalar2=b_c[:, g : g + 1],
                            op0=ALU.mult, op1=ALU.add,
                        )
                        nc.sync.dma_start(out=yr[:, g, s:e], in_=y_sb[:, g, s:e])

    _finalize(nc)
    return nc


def _get_nc(dbg=False):
    key = ("ncd" if dbg else "nc")
    if key not in _CACHE:
        _CACHE[key] = _build(dbg)
    return _CACHE[key]


def make_in_maps(x, context, Wq, bq, Wk, bk, Wv, bv, Wo, bo, gamma, beta):
    x = np.asarray(x, np.float32)
    context = np.asarray(context, np.float32)
    xr = np.ascontiguousarray(x.reshape(B, C, HW).astype(np.float16))
    cr = np.ascontiguousarray(context.reshape(B, C, HW).astype(np.float16))

    gsel = np.zeros((128, 16), np.float32)
    gsel[np.arange(128), np.arange(128) // GSIZE] = 1.0 / GN_COUNT

    Wo32 = np.asarray(Wo, np.float32)
    bo_eff = np.asarray(bo, np.float32) + Wo32 @ np.asarray(bv, np.float32)

    shared = {
        "wqT": np.ascontiguousarray(np.asarray(Wq, np.float32).T.astype(np.float16)),
        "wkT": np.ascontiguousarray(np.asarray(Wk, np.float32).T.astype(np.float16)),
        "wvT": np.ascontiguousarray(np.asarray(Wv, np.float32).T.astype(np.float16)),
        "woT": np.ascontiguousarray(Wo32.T.astype(np.float16)),
        "bq": np.asarray(bq, np.float32).reshape(C, 1),
        "bk": np.asarray(bk, np.float32).reshape(C, 1),
        "bo": bo_eff.reshape(C, 1),
        "gamma": np.asarray(gamma, np.float32).reshape(C, 1),
        "beta": np.asarray(beta, np.float32).reshape(C, 1),
        "gsel": gsel,
        "gselT": np.ascontiguousarray((gsel.T != 0).astype(np.float32)),
    }
    in_maps = []
    for core in range(8):
        b, half = core // 2, core % 2
        m = dict(shared)
        m["xh"] = np.ascontiguousarray(xr[b, :, half * NHALF : (half + 1) * NHALF])
        m["ctx"] = cr[b]
        in_maps.append(m)
    return in_maps


def kernel(x, context, Wq, bq, Wk, bk, Wv, bv, Wo, bo, gamma, beta):
    in_maps = make_in_maps(
        x, context, Wq, bq, Wk, bk, Wv, bv, Wo, bo, gamma, beta
    )
    x = np.asarray(x, np.float32)

    nc = _get_nc()
    res = run_bass_kernel_spmd(nc, in_maps, core_ids=list(range(8)))

    out = np.empty((B, C, HW), np.float32)
    for core in range(8):
        b, half = core // 2, core % 2
        out[b, :, half * NHALF : (half + 1) * NHALF] = res.results[core]["yh"]
    return out.reshape(x.shape)


# revision 11
# speedup vs baseline: 1.1397x; 1.0070x over previous
"""Cross-attention + output projection + residual + GroupNorm on 8 NeuronCores.

Problem (hardcoded): B=4, C=256, H=W=48 (N=2304 pixels), 4 heads x 64 dim,
GroupNorm with 32 groups of 8 channels, eps=1e-5.

Sharding: 2 cores per batch element; each core handles one half of the
query pixels (1152) for all 4 heads.  K/V are computed for the full pixel
range on both cores of a pair (duplicated, cheap).  The only cross-core
communication is a tiny AllReduce of per-group (sum, sumsq) GroupNorm
partial statistics between the two cores of each pair.

v5 design (baseline was 169us, ACT-exp-bound and serialized):
  * softmax exp runs on TWO engines concurrently, split at the PSUM bank
    boundary of each score tile:
      ACT:  ex[0:512]    = Exp(scale*s)        from sc_a (bank 0)
      DVE:  ex[512:1152] = bits_fp16(int16(A*s+B))  from sc_b (banks 1-2)
    The DVE path is a Schraudolph 2^x bit trick (A=1024*log2(e)*scale,
    B=15*1024-45, HW rounds to nearest); ~3% rel err on attention weights
    lands ~8e-4 absmax-relative on the final output.
  * score matmuls are row-tiled: each head's 64-dim contraction runs as
    two concurrent 64-row PE tiles (partitions 0-63 / 64-127) fed by
    partition-swapped duplicates q2/k2 (SBUF->SBUF DMA).
  * one 8-bank PSUM pool for proj/V/scores/AV so the first scores issue
    right after the K(jh0) projection; V rides inside head-0's slots.
  * AV accumulates in-slot (lag 2; head 0 lag 6) into 2 packed banks,
    bank-first matmul start=True instead of memsets.
  * tail: y = (Wo@ao + bo) + x fused with the channel sums in one
    scalar_tensor_tensor; per-group totals computed BEFORE the pair
    AllReduce (128-byte payload); a dummy AllReduce early in the kernel
    absorbs the collective warm-up latency.
"""

import sys

if "/opt/trn_rl_repo" not in sys.path:
    sys.path.insert(0, "/opt/trn_rl_repo")

import math
import os as _os

import ml_dtypes
import numpy as np

import concourse.bass as bass
import concourse.mybir as mybir
import concourse.tile as tile
from concourse import bacc
from concourse.bass_utils import run_bass_kernel_spmd

F32 = mybir.dt.float32
BF16 = mybir.dt.bfloat16
F16 = mybir.dt.float16
I16 = mybir.dt.int16
AF = mybir.ActivationFunctionType
ALU = mybir.AluOpType
AXF = mybir.AxisListType.X

B, C, HW = 4, 256, 2304
NH, HD = 4, 64
NHALF = HW // 2  # 1152 query pixels per core
NJT = HW // 128  # 18 key tiles of 128
NIT = NHALF // 128  # 9 query tiles of 128
SCALE = HD ** -0.5  # 0.125
GSIZE = 8  # channels per GroupNorm group
EPS = 1e-5
GN_COUNT = GSIZE * HW  # elements per group per batch (after pair AllReduce)

# Schraudolph fp16 exp: exp(SCALE*s) ~= bits_fp16(int16(EXP_A*s + EXP_B))
EXP_A = 1024.0 * math.log2(math.e) * SCALE
EXP_B = 15.0 * 1024.0 - 45.0

NA = 5  # query tiles in AV accumulator bank A (rest in bank B)


def _flag(name, default):
    v = _os.environ.get(name)
    return default if v is None else v == "1"


USE_DVE_EXP = _flag("CA_DVE_EXP", True)  # Schraudolph exp half on DVE
USE_CWARM = _flag("CA_CWARM", True)  # dummy early AllReduce

_CACHE = {}


def _mm_slices(total, step=512):
    return [(s, min(s + step, total)) for s in range(0, total, step)]


def _finalize(nc):
    """compile() leaves 3+-wait Matmults that walrus rejects ("Too many sync
    wait commands" on the S3_LW struct); a second compile pass — run here via
    finalize() — splits them onto EventSemaphores.  Verify that it worked."""
    nc.compile()
    nc.finalize()
    for fn in nc.m.functions:
        for bb in fn.blocks:
            for inst in bb.instructions:
                si = inst.sync_info
                if isinstance(inst, mybir.InstMatmult) and si is not None:
                    assert len(si.on_wait or []) <= 2, (inst.name, si.on_wait)


def _build(dbg=False):
    nc = bacc.Bacc("TRN2", target_bir_lowering=False, debug=False, num_devices=8)

    xh_d = nc.dram_tensor("xh", [C, NHALF], F16, kind="ExternalInput").ap()
    ctx_d = nc.dram_tensor("ctx", [C, HW], F16, kind="ExternalInput").ap()
    w_d = {
        nm: nc.dram_tensor(nm, [C, C], F16, kind="ExternalInput").ap()
        for nm in ("wqT", "wkT", "wvT", "woT")
    }
    b_d = {
        nm: nc.dram_tensor(nm, [C, 1], F32, kind="ExternalInput").ap()
        for nm in ("bq", "bk", "bo", "gamma", "beta")
    }
    gsel_d = nc.dram_tensor("gsel", [128, 16], F32, kind="ExternalInput").ap()
    gselT_d = nc.dram_tensor("gselT", [16, 128], F32, kind="ExternalInput").ap()
    yh_d = nc.dram_tensor("yh", [C, NHALF], F32, kind="ExternalOutput").ap()
    if dbg:
        dbg_q = nc.dram_tensor("dbg_q", [C, NHALF], F16, kind="ExternalOutput").ap()
        dbg_k = nc.dram_tensor("dbg_k", [C, HW], F16, kind="ExternalOutput").ap()
        dbg_v = nc.dram_tensor("dbg_v", [128, NJT, NH * (HD + 1)], F16, kind="ExternalOutput").ap()
        dbg_ao = nc.dram_tensor("dbg_ao", [C, NHALF], F16, kind="ExternalOutput").ap()
        dbg_y = nc.dram_tensor("dbg_y", [C, NHALF], F32, kind="ExternalOutput").ap()
        dbg_st = nc.dram_tensor("dbg_st", [C, 2], F32, kind="ExternalOutput").ap()
    warm_d = nc.dram_tensor("warm_out", [16, 1], F32, kind="ExternalOutput").ap()
    warm2_d = nc.dram_tensor("warm2_out", [16, 1], F32, kind="ExternalOutput").ap()

    with tile.TileContext(nc) as tc:
        with (
            tc.tile_pool(name="const", bufs=1) as const,
            tc.tile_pool(name="main", bufs=1) as main,
            tc.tile_pool(name="small", bufs=4) as small,
            tc.tile_pool(name="dram", bufs=2, space="DRAM") as dram,
        ):
            # ---- constants (DMA order = consumption order) ----
            w_sb = {}
            for nm in ("wqT", "wkT", "wvT", "woT"):
                w_sb[nm] = const.tile([128, 2, C], F16, tag=nm, name=nm)
            b_sb = {}
            for nm in ("bq", "bk", "bo", "gamma", "beta"):
                b_sb[nm] = const.tile([128, 2], F32, tag=nm, name=nm)

            xh_sb = main.tile([128, 2, NHALF], F16, tag="xh")
            ctx_sb = main.tile([128, 2, HW], F16, tag="ctx")
            nc.sync.dma_start(
                out=w_sb["wqT"], in_=w_d["wqT"].rearrange("(k p) o -> p k o", p=128)
            )
            nc.sync.dma_start(out=xh_sb, in_=xh_d.rearrange("(k p) i -> p k i", p=128))
            nc.sync.dma_start(
                out=w_sb["wkT"], in_=w_d["wkT"].rearrange("(k p) o -> p k o", p=128)
            )
            # biases are tiny and gate the q/k copies -- issue them early
            for nm in ("bq", "bk", "bo"):
                nc.sync.dma_start(
                    out=b_sb[nm],
                    in_=b_d[nm].rearrange("(k p) one -> p (k one)", p=128),
                )
            ctx_r = ctx_d.rearrange("(k p) j -> p k j", p=128)
            for jq in range(4):
                s0, s1 = jq * (HW // 4), (jq + 1) * (HW // 4)
                nc.sync.dma_start(out=ctx_sb[:, :, s0:s1], in_=ctx_r[:, :, s0:s1])
            nc.sync.dma_start(
                out=w_sb["wvT"], in_=w_d["wvT"].rearrange("(k p) o -> p k o", p=128)
            )
            for nm in ("gamma", "beta"):
                nc.sync.dma_start(
                    out=b_sb[nm],
                    in_=b_d[nm].rearrange("(k p) one -> p (k one)", p=128),
                )
            gsel_sb = const.tile([128, 16], F32, tag="gsel")
            nc.sync.dma_start(out=gsel_sb, in_=gsel_d)
            gselT_sb = const.tile([16, 128], F32, tag="gselT")
            nc.sync.dma_start(out=gselT_sb, in_=gselT_d)
            nc.sync.dma_start(
                out=w_sb["woT"], in_=w_d["woT"].rearrange("(k p) o -> p k o", p=128)
            )

            # warm the Exp ACT table while input DMAs run
            eps_sb = const.tile([16, 1], F32, tag="eps")
            nc.vector.memset(eps_sb, EPS)
            warm_e = small.tile([16, 1], F32, tag="warme", bufs=1)
            nc.scalar.activation(out=warm_e, in_=eps_sb, func=AF.Exp)
            nc.sync.dma_start(out=warm2_d, in_=warm_e)

            q_sb = main.tile([128, 2, NHALF], F16, tag="q")
            q2_sb = main.tile([128, 2, NHALF], F16, tag="q2")
            k_sb = main.tile([128, 2, HW], F16, tag="k")
            k2_sb = main.tile([128, 2, HW], F16, tag="k2")
            vT_sb = main.tile([128, NJT, NH * (HD + 1)], F16, tag="vT")
            ao_sb = main.tile([128, 2, NHALF], F16, tag="ao")
            aon_sb = main.tile([128, 2, NIT, 128], F16, tag="aon")
            y_sb = main.tile([128, 2, NHALF], F32, tag="y")
            scr_sb = main.tile([128, NHALF], F16, tag="scr")

            # ones columns of vT (one per head, strided over j-tiles)
            for h in range(NH):
                c0 = h * (HD + 1) + HD
                nc.vector.memset(vT_sb[:, :, c0 : c0 + 1], 1.0)

            if USE_CWARM:
                # dummy tiny AllReduce overlapped with attention -- absorbs
                # first-use collective latency for the GN-tail AllReduce.
                cwarm_sb = small.tile([16, 1], F32, tag="cwarm", bufs=1)
                nc.vector.memset(cwarm_sb, 1.0)
                cw_in = dram.tile([16, 1], F32, tag="cwin", bufs=1)
                cw_out = dram.tile([16, 1], F32, tag="cwout", bufs=1)
                nc.sync.dma_start(out=cw_in, in_=cwarm_sb)
                nc.gpsimd.collective_compute(
                    "AllReduce", ALU.add,
                    replica_groups=[[0, 1], [2, 3], [4, 5], [6, 7]],
                    ins=[cw_in.opt()], outs=[cw_out.opt()],
                )

            # ---- projections + V in the pp PSUM pool (8 banks) ----
            with (
                tc.tile_pool(name="pp", bufs=2, space="PSUM") as pp,
            ):
                def proj(dst, wT, src, src_off, bias, nm):
                    """dst[0:1152] = wT.T @ src[src_off:+1152] + bias, via a
                    3-bank psum tile; copy+bias split ACT (first 512) / DVE."""
                    ps = pp.tile([128, NHALF], F32, tag="qk", name=f"ps_{nm}")
                    for kk in range(2):
                        for s, e in _mm_slices(NHALF):
                            nc.tensor.matmul(
                                ps[:, s:e], wT[:, kk, :],
                                src[:, kk, src_off + s : src_off + e],
                                start=(kk == 0), stop=(kk == 1),
                            )
                    nc.scalar.activation(out=dst[:, 0:512], in_=ps[:, 0:512],
                                         func=AF.Identity, bias=bias)
                    nc.vector.tensor_scalar_add(out=dst[:, 512:NHALF],
                                                in0=ps[:, 512:NHALF],
                                                scalar1=bias)

                for g in range(2):
                    proj(q_sb[:, g, :], w_sb["wqT"][:, :, g * 128 : (g + 1) * 128],
                         xh_sb, 0, b_sb["bq"][:, g : g + 1], f"q{g}")
                nc.sync.dma_start(out=q2_sb[0:64, :, :], in_=q_sb[64:128, :, :])
                nc.sync.dma_start(out=q2_sb[64:128, :, :], in_=q_sb[0:64, :, :])
                for jh in range(2):
                    for g in range(2):
                        proj(k_sb[:, g, jh * NHALF : (jh + 1) * NHALF],
                             w_sb["wkT"][:, :, g * 128 : (g + 1) * 128],
                             ctx_sb, jh * NHALF, b_sb["bk"][:, g : g + 1],
                             f"k{jh}{g}")
                    nc.sync.dma_start(
                        out=k2_sb[0:64, :, jh * NHALF : (jh + 1) * NHALF],
                        in_=k_sb[64:128, :, jh * NHALF : (jh + 1) * NHALF],
                    )
                    nc.sync.dma_start(
                        out=k2_sb[64:128, :, jh * NHALF : (jh + 1) * NHALF],
                        in_=k_sb[0:64, :, jh * NHALF : (jh + 1) * NHALF],
                    )

                # V: vT[j, c] = (ctx.T @ WvT)[j, c] per j-tile; copies split
                # across DVE / ACT by jt parity
                for jt in range(NJT):
                    vp = pp.tile([128, C], F32, tag="vp", name=f"vp{jt}")
                    for kk in range(2):
                        nc.tensor.matmul(
                            vp, ctx_sb[:, kk, jt * 128 : (jt + 1) * 128],
                            w_sb["wvT"][:, kk, :],
                            start=(kk == 0), stop=(kk == 1),
                        )
                    vdst = (
                        vT_sb[:, jt, :]
                        .rearrange("p (h e) -> p h e", e=HD + 1)[:, :, :HD]
                    )
                    vsrc = vp.rearrange("p (h d) -> p h d", d=HD)
                    if jt % 2 == 0:
                        nc.vector.tensor_copy(out=vdst, in_=vsrc)
                    else:
                        nc.scalar.activation(out=vdst, in_=vsrc, func=AF.Identity)

            # ---- attention: per head, AV trails scores/exp by 2 slots ----
            with (
                tc.tile_pool(name="expp", bufs=8) as expp,
                tc.tile_pool(name="scp", bufs=2, space="PSUM") as scp,
                tc.tile_pool(name="avp", bufs=1, space="PSUM") as avp,
            ):
                ex_tiles = {}
                av_banks = {}

                def issue_scores(h, jt):
                    # two PSUM tiles per slot so the ACT / DVE exp halves
                    # release their buffers independently:
                    #   sc_a [128,512] (1 bank):  q[0:512]    on rows 0-63
                    #   sc_b [128,640] (2 banks): q[512:1024] on rows 64-127
                    #                             q[1024:1152] on rows 0-63
                    g, ph = h // 2, h % 2
                    qlo, klo = (q_sb, k_sb) if ph == 0 else (q2_sb, k2_sb)
                    qhi, khi = (q2_sb, k2_sb) if ph == 0 else (q_sb, k_sb)
                    sca = scp.tile([128, 512], F32, tag="sca", name=f"sca{h}_{jt}")
                    scb = scp.tile([128, 640], F32, tag="scb", name=f"scb{h}_{jt}")
                    j0, j1 = jt * 128, (jt + 1) * 128
                    nc.tensor.matmul(
                        scb[:, 512:640], klo[0:64, g, j0:j1],
                        qlo[0:64, g, 1024:1152],
                        start=True, stop=True,
                    )
                    nc.tensor.matmul(
                        scb[:, 0:512], khi[64:128, g, j0:j1],
                        qhi[64:128, g, 512:1024],
                        start=True, stop=True,
                    )
                    nc.tensor.matmul(
                        sca[:, 0:512], klo[0:64, g, j0:j1],
                        qlo[0:64, g, 0:512],
                        start=True, stop=True,
                    )
                    return sca, scb

                def issue_exp(h, jt, sc):
                    sca, scb = sc
                    ex = expp.tile([128, NHALF], F16, tag="exp", name=f"ex{h}_{jt}")
                    nc.scalar.activation(out=ex[:, 0:512], in_=sca,
                                         func=AF.Exp, scale=SCALE)
                    if USE_DVE_EXP:
                        nc.vector.tensor_scalar(
                            out=ex[:, 512:NHALF].bitcast(I16), in0=scb,
                            scalar1=EXP_A, scalar2=EXP_B,
                            op0=ALU.mult, op1=ALU.add,
                        )
                    else:
                        nc.scalar.activation(out=ex[:, 512:NHALF], in_=scb,
                                             func=AF.Exp, scale=SCALE)
                    ex_tiles[h, jt] = ex

                def issue_av(h, jt):
                    ava, avb = av_banks[h]
                    col = h * (HD + 1)
                    for it in range(NIT):
                        dst = (
                            ava[:, it * (HD + 1) : (it + 1) * (HD + 1)]
                            if it < NA
                            else avb[:, (it - NA) * (HD + 1) : (it - NA + 1) * (HD + 1)]
                        )
                        nc.tensor.matmul(
                            dst,
                            ex_tiles[h, jt][:, it * 128 : (it + 1) * 128],
                            vT_sb[:, jt, col : col + HD + 1],
                            start=(jt == 0 and it in (0, NA)),
                            stop=(jt == NJT - 1),
                            skip_group_check=True,
                        )
                    del ex_tiles[h, jt]

                def issue_norm(h):
                    ava, avb = av_banks[h]
                    g, ph = h // 2, h % 2
                    rdena = small.tile([128, NA], F32, tag="rdena", bufs=2)
                    rdenb = small.tile([128, NIT - NA], F32, tag="rdenb", bufs=2)
                    nc.vector.reciprocal(out=rdena, in_=ava[:, HD :: HD + 1])
                    nc.vector.reciprocal(out=rdenb, in_=avb[:, HD :: HD + 1])
                    for it in range(NIT):
                        src = (
                            ava[:, it * (HD + 1) : it * (HD + 1) + HD]
                            if it < NA
                            else avb[:, (it - NA) * (HD + 1) : (it - NA) * (HD + 1) + HD]
                        )
                        rd = (
                            rdena[:, it : it + 1] if it < NA
                            else rdenb[:, it - NA : it - NA + 1]
                        )
                        nc.vector.tensor_scalar_mul(
                            out=aon_sb[:, g, it, ph * HD : (ph + 1) * HD],
                            in0=src, scalar1=rd,
                        )

                for h in range(NH):
                    av_banks[h] = (
                        avp.tile([128, NA * (HD + 1)], F32, tag="ava",
                                 name=f"ava{h}"),
                        avp.tile([128, (NIT - NA) * (HD + 1)], F32, tag="avb",
                                 name=f"avb{h}"),
                    )
                    for jt in range(NJT):
                        issue_exp(h, jt, issue_scores(h, jt))
                        if jt >= 2:
                            issue_av(h, jt - 2)
                    for jt in range(NJT - 2, NJT):
                        issue_av(h, jt)
                    issue_norm(h)
                    if h % 2 == 1:  # both heads of the g-half staged
                        nc.sync.dma_start(
                            out=ao_sb[:, h // 2, :]
                            .rearrange("p (t i) -> p t i", i=128),
                            in_=aon_sb[:, h // 2, :, :],
                            transpose=True,
                        )

            if dbg:
                nc.sync.dma_start(out=dbg_q.rearrange("(k p) i -> p k i", p=128), in_=q_sb)
                nc.sync.dma_start(out=dbg_k.rearrange("(k p) i -> p k i", p=128), in_=k_sb)
                nc.sync.dma_start(out=dbg_v, in_=vT_sb)
                nc.sync.dma_start(out=dbg_ao.rearrange("(k p) i -> p k i", p=128), in_=ao_sb)

            # Sqrt lives in a different ACT table set than Exp: warm the sqrt
            # table during the Wo/stats phase (input dep on the transposed ao
            # keeps the Tile scheduler from hoisting it into the exp stream).
            warm = small.tile([16, 1], F32, tag="warm", bufs=1)
            nc.scalar.sqrt(out=warm, in_=ao_sb[0:16, 1, 0:1])
            nc.sync.dma_start(out=warm_d, in_=warm)

            # ---- output projection + residual + GroupNorm ----
            with (
                tc.tile_pool(name="wop", bufs=2, space="PSUM") as wop,
                tc.tile_pool(name="gnp", bufs=1, space="PSUM") as gnp,
            ):
                st4 = small.tile([128, 2, 2], F32, tag="st4", bufs=1)
                wps = []
                for kk in range(2):
                    for g in range(2):
                        if kk == 0:
                            wps.append(wop.tile([128, NHALF], F32, tag="wo",
                                                name=f"wo{g}"))
                        lhsT = w_sb["woT"][:, kk, g * 128 : (g + 1) * 128]
                        for s, e in _mm_slices(NHALF):
                            nc.tensor.matmul(wps[g][:, s:e], lhsT,
                                             ao_sb[:, kk, s:e],
                                             start=(kk == 0), stop=(kk == 1))
                for g in range(2):
                    # y = (wo + bo) + x, channel sums ride in accum_out
                    nc.vector.scalar_tensor_tensor(
                        out=y_sb[:, g, :], in0=wps[g],
                        scalar=b_sb["bo"][:, g : g + 1],
                        in1=xh_sb[:, g, :],
                        op0=ALU.add, op1=ALU.add,
                        accum_out=st4[:, g, 0:1],
                    )
                    nc.scalar.activation(
                        out=scr_sb, in_=y_sb[:, g, :], func=AF.Square,
                        accum_out=st4[:, g, 1:2],
                    )

                # local per-group (sum, sumsq) BEFORE the AllReduce
                lgt = small.tile([16, 2, 2], F32, tag="lgt", bufs=1)
                for kk in range(2):
                    gp = gnp.tile([16, 2], F32, tag="gp")
                    nc.tensor.matmul(gp, gsel_sb, st4[:, kk, :],
                                     start=True, stop=True)
                    nc.vector.tensor_copy(out=lgt[:, kk, :], in_=gp)

                # pair AllReduce of per-group (sum, sumsq): 2x 128B
                gn_in = dram.tile([16, 4], F32, tag="gnin", bufs=1)
                gn_out = dram.tile([16, 4], F32, tag="gnout", bufs=1)
                nc.sync.dma_start(
                    out=gn_in, in_=lgt.rearrange("p a b -> p (a b)")
                )
                nc.gpsimd.collective_compute(
                    "AllReduce", ALU.add,
                    replica_groups=[[0, 1], [2, 3], [4, 5], [6, 7]],
                    ins=[gn_in.opt()], outs=[gn_out.opt()],
                )
                gtot = small.tile([16, 2, 2], F32, tag="gtot", bufs=1)
                nc.sync.dma_start(
                    out=gtot.rearrange("p a b -> p (a b)"), in_=gn_out
                )

                mean_g = gtot[:, :, 0]
                var_g = small.tile([16, 2], F32, tag="varg", bufs=1)
                nc.vector.tensor_copy(out=var_g, in_=gtot[:, :, 1])
                m2 = small.tile([16, 2], F32, tag="m2", bufs=1)
                nc.vector.tensor_mul(out=m2, in0=mean_g, in1=mean_g)
                nc.vector.tensor_tensor(out=var_g, in0=var_g, in1=m2,
                                        op=ALU.subtract)
                nc.scalar.activation(out=var_g, in_=var_g, func=AF.Sqrt,
                                     bias=eps_sb)
                nc.vector.reciprocal(out=var_g, in_=var_g)  # rstd [16, 2]

                # broadcast group stats to channels, fold gamma/beta into A,B
                mean_c = small.tile([128, 2], F32, tag="meanc", bufs=1)
                rstd_c = small.tile([128, 2], F32, tag="rstdc", bufs=1)
                for src, dst in ((mean_g, mean_c), (var_g, rstd_c)):
                    gp = gnp.tile([128, 2], F32, tag="gb")
                    nc.tensor.matmul(gp, gselT_sb, src, start=True, stop=True)
                    nc.vector.tensor_copy(out=dst, in_=gp)
                a_c = small.tile([128, 2], F32, tag="ac", bufs=1)
                b_c = small.tile([128, 2], F32, tag="bc", bufs=1)
                nc.vector.tensor_mul(out=a_c, in0=rstd_c, in1=b_sb["gamma"])
                nc.vector.tensor_mul(out=b_c, in0=mean_c, in1=a_c)
                nc.vector.tensor_tensor(out=b_c, in0=b_sb["beta"], in1=b_c,
                                        op=ALU.subtract)

                if dbg:
                    nc.sync.dma_start(out=dbg_y.rearrange("(k p) i -> p k i", p=128), in_=y_sb)
                    nc.sync.dma_start(out=dbg_st.rearrange("(k p) s -> p k s", p=128)[:, :, 0:1], in_=st4[:, :, 0:1])
                    nc.sync.dma_start(out=dbg_st.rearrange("(k p) s -> p k s", p=128)[:, :, 1:2], in_=st4[:, :, 1:2])
                yr = yh_d.rearrange("(k p) i -> p k i", p=128)
                for g in range(2):
                    for ci, (s, e) in enumerate(_mm_slices(NHALF)):
                        nc.vector.tensor_scalar(
                            out=y_sb[:, g, s:e], in0=y_sb[:, g, s:e],
                            scalar1=a_c[:, g : g + 1], scalar2=b_c[:, g : g + 1],
                            op0=ALU.mult, op1=ALU.add,
                        )
                        nc.sync.dma_start(out=yr[:, g, s:e], in_=y_sb[:, g, s:e])

    _finalize(nc)
    return nc


def _get_nc(dbg=False):
    key = "ncd" if dbg else "nc"
    if key not in _CACHE:
        _CACHE[key] = _build(dbg)
    return _CACHE[key]


def make_in_maps(x, context, Wq, bq, Wk, bk, Wv, bv, Wo, bo, gamma, beta):
    x = np.asarray(x, np.float32)
    context = np.asarray(context, np.float32)
    xr = np.ascontiguousarray(x.reshape(B, C, HW).astype(np.float16))
    cr = np.ascontiguousarray(context.reshape(B, C, HW).astype(np.float16))

    gsel = np.zeros((128, 16), np.float32)
    gsel[np.arange(128), np.arange(128) // GSIZE] = 1.0 / GN_COUNT

    Wo32 = np.asarray(Wo, np.float32)
    bo_eff = np.asarray(bo, np.float32) + Wo32 @ np.asarray(bv, np.float32)

    shared = {
        "wqT": np.ascontiguousarray(np.asarray(Wq, np.float32).T.astype(np.float16)),
        "wkT": np.ascontiguousarray(np.asarray(Wk, np.float32).T.astype(np.float16)),
        "wvT": np.ascontiguousarray(np.asarray(Wv, np.float32).T.astype(np.float16)),
        "woT": np.ascontiguousarray(Wo32.T.astype(np.float16)),
        "bq": np.asarray(bq, np.float32).reshape(C, 1),
        "bk": np.asarray(bk, np.float32).reshape(C, 1),
        "bo": bo_eff.reshape(C, 1),
        "gamma": np.asarray(gamma, np.float32).reshape(C, 1),
        "beta": np.asarray(beta, np.float32).reshape(C, 1),
        "gsel": gsel,
        "gselT": np.ascontiguousarray((gsel.T != 0).astype(np.float32)),
    }
    in_maps = []
    for core in range(8):
        b, half = core // 2, core % 2
        m = dict(shared)
        m["xh"] = np.ascontiguousarray(xr[b, :, half * NHALF : (half + 1) * NHALF])
        m["ctx"] = cr[b]
        in_maps.append(m)
    return in_maps


def kernel(x, context, Wq, bq, Wk, bk, Wv, bv, Wo, bo, gamma, beta):
    in_maps = make_in_maps(
        x, context, Wq, bq, Wk, bk, Wv, bv, Wo, bo, gamma, beta
    )
    x = np.asarray(x, np.float32)

    nc = _get_nc()
    res = run_bass_kernel_spmd(nc, in_maps, core_ids=list(range(8)))

    out = np.empty((B, C, HW), np.float32)
    for core in range(8):
        b, half = core // 2, core % 2
        out[b, :, half * NHALF : (half + 1) * NHALF] = res.results[core]["yh"]
    return out.reshape(x.shape)


# revision 12
# speedup vs baseline: 1.1483x; 1.0076x over previous
"""Cross-attention + output projection + residual + GroupNorm on 8 NeuronCores.

Problem (hardcoded): B=4, C=256, H=W=48 (N=2304 pixels), 4 heads x 64 dim,
GroupNorm with 32 groups of 8 channels, eps=1e-5.

Sharding: 2 cores per batch element; each core handles one half of the
query pixels (1152) for all 4 heads.  K/V are computed for the full pixel
range on both cores of a pair (duplicated, cheap).  The only cross-core
communication is a tiny AllReduce of per-group (sum, sumsq) GroupNorm
partial statistics between the two cores of each pair.

v5 design (baseline was 169us, ACT-exp-bound and serialized):
  * softmax exp runs on TWO engines concurrently, split at the PSUM bank
    boundary of each score tile:
      ACT:  ex[0:512]    = Exp(scale*s)        from sc_a (bank 0)
      DVE:  ex[512:1152] = bits_fp16(int16(A*s+B))  from sc_b (banks 1-2)
    The DVE path is a Schraudolph 2^x bit trick (A=1024*log2(e)*scale,
    B=15*1024-45, HW rounds to nearest); ~3% rel err on attention weights
    lands ~8e-4 absmax-relative on the final output.
  * score matmuls are row-tiled: each head's 64-dim contraction runs as
    two concurrent 64-row PE tiles (partitions 0-63 / 64-127) fed by
    partition-swapped duplicates q2/k2 (SBUF->SBUF DMA).
  * one 8-bank PSUM pool for proj/V/scores/AV so the first scores issue
    right after the K(jh0) projection; V rides inside head-0's slots.
  * AV accumulates in-slot (lag 2; head 0 lag 6) into 2 packed banks,
    bank-first matmul start=True instead of memsets.
  * tail: y = (Wo@ao + bo) + x fused with the channel sums in one
    scalar_tensor_tensor; per-group totals computed BEFORE the pair
    AllReduce (128-byte payload); a dummy AllReduce early in the kernel
    absorbs the collective warm-up latency.
"""

import sys

if "/opt/trn_rl_repo" not in sys.path:
    sys.path.insert(0, "/opt/trn_rl_repo")

import math
import os as _os

import ml_dtypes
import numpy as np

import concourse.bass as bass
import concourse.mybir as mybir
import concourse.tile as tile
from concourse import bacc
from concourse.bass_utils import run_bass_kernel_spmd

F32 = mybir.dt.float32
BF16 = mybir.dt.bfloat16
F16 = mybir.dt.float16
I16 = mybir.dt.int16
AF = mybir.ActivationFunctionType
ALU = mybir.AluOpType
AXF = mybir.AxisListType.X

B, C, HW = 4, 256, 2304
NH, HD = 4, 64
NHALF = HW // 2  # 1152 query pixels per core
NJT = HW // 128  # 18 key tiles of 128
NIT = NHALF // 128  # 9 query tiles of 128
SCALE = HD ** -0.5  # 0.125
GSIZE = 8  # channels per GroupNorm group
EPS = 1e-5
GN_COUNT = GSIZE * HW  # elements per group per batch (after pair AllReduce)

# Schraudolph fp16 exp: exp(SCALE*s) ~= bits_fp16(int16(EXP_A*s + EXP_B))
EXP_A = 1024.0 * math.log2(math.e) * SCALE
EXP_B = 15.0 * 1024.0 - 45.0

NA = 5  # query tiles in AV accumulator bank A (rest in bank B)


def _flag(name, default):
    v = _os.environ.get(name)
    return default if v is None else v == "1"


USE_DVE_EXP = _flag("CA_DVE_EXP", True)  # Schraudolph exp half on DVE
USE_CWARM = _flag("CA_CWARM", True)  # dummy early AllReduce

_CACHE = {}


def _mm_slices(total, step=512):
    return [(s, min(s + step, total)) for s in range(0, total, step)]


def _finalize(nc):
    """compile() leaves 3+-wait Matmults that walrus rejects ("Too many sync
    wait commands" on the S3_LW struct); a second compile pass — run here via
    finalize() — splits them onto EventSemaphores.  Verify that it worked."""
    nc.compile()
    nc.finalize()
    for fn in nc.m.functions:
        for bb in fn.blocks:
            for inst in bb.instructions:
                si = inst.sync_info
                if isinstance(inst, mybir.InstMatmult) and si is not None:
                    assert len(si.on_wait or []) <= 2, (inst.name, si.on_wait)


def _build(dbg=False):
    nc = bacc.Bacc("TRN2", target_bir_lowering=False, debug=False, num_devices=8)

    xh_d = nc.dram_tensor("xh", [C, NHALF], F16, kind="ExternalInput").ap()
    ctx_d = nc.dram_tensor("ctx", [C, HW], F16, kind="ExternalInput").ap()
    w_d = {
        nm: nc.dram_tensor(nm, [C, C], F16, kind="ExternalInput").ap()
        for nm in ("wqT", "wkT", "wvT", "woT")
    }
    b_d = {
        nm: nc.dram_tensor(nm, [C, 1], F32, kind="ExternalInput").ap()
        for nm in ("bq", "bk", "bo", "gamma", "beta")
    }
    gsel_d = nc.dram_tensor("gsel", [128, 16], F32, kind="ExternalInput").ap()
    gselT_d = nc.dram_tensor("gselT", [16, 128], F32, kind="ExternalInput").ap()
    yh_d = nc.dram_tensor("yh", [C, NHALF], F32, kind="ExternalOutput").ap()
    if dbg:
        dbg_q = nc.dram_tensor("dbg_q", [C, NHALF], F16, kind="ExternalOutput").ap()
        dbg_k = nc.dram_tensor("dbg_k", [C, HW], F16, kind="ExternalOutput").ap()
        dbg_v = nc.dram_tensor("dbg_v", [128, NJT, NH * (HD + 1)], F16, kind="ExternalOutput").ap()
        dbg_ao = nc.dram_tensor("dbg_ao", [C, NHALF], F16, kind="ExternalOutput").ap()
        dbg_y = nc.dram_tensor("dbg_y", [C, NHALF], F32, kind="ExternalOutput").ap()
        dbg_st = nc.dram_tensor("dbg_st", [C, 2], F32, kind="ExternalOutput").ap()
    warm_d = nc.dram_tensor("warm_out", [16, 1], F32, kind="ExternalOutput").ap()
    warm2_d = nc.dram_tensor("warm2_out", [16, 1], F32, kind="ExternalOutput").ap()

    with tile.TileContext(nc) as tc:
        with (
            tc.tile_pool(name="const", bufs=1) as const,
            tc.tile_pool(name="main", bufs=1) as main,
            tc.tile_pool(name="small", bufs=4) as small,
            tc.tile_pool(name="dram", bufs=2, space="DRAM") as dram,
        ):
            # ---- constants (DMA order = consumption order) ----
            w_sb = {}
            for nm in ("wqT", "wkT", "wvT", "woT"):
                w_sb[nm] = const.tile([128, 2, C], F16, tag=nm, name=nm)
            b_sb = {}
            for nm in ("bq", "bk", "bo", "gamma", "beta"):
                b_sb[nm] = const.tile([128, 2], F32, tag=nm, name=nm)

            xh_sb = main.tile([128, 2, NHALF], F16, tag="xh")
            ctx_sb = main.tile([128, 2, HW], F16, tag="ctx")
            nc.sync.dma_start(
                out=w_sb["wqT"], in_=w_d["wqT"].rearrange("(k p) o -> p k o", p=128)
            )
            nc.sync.dma_start(out=xh_sb, in_=xh_d.rearrange("(k p) i -> p k i", p=128))
            nc.sync.dma_start(
                out=w_sb["wkT"], in_=w_d["wkT"].rearrange("(k p) o -> p k o", p=128)
            )
            # biases are tiny and gate the q/k copies -- issue them early
            for nm in ("bq", "bk", "bo"):
                nc.sync.dma_start(
                    out=b_sb[nm],
                    in_=b_d[nm].rearrange("(k p) one -> p (k one)", p=128),
                )
            ctx_r = ctx_d.rearrange("(k p) j -> p k j", p=128)
            for jq in range(4):
                s0, s1 = jq * (HW // 4), (jq + 1) * (HW // 4)
                nc.sync.dma_start(out=ctx_sb[:, :, s0:s1], in_=ctx_r[:, :, s0:s1])
            nc.sync.dma_start(
                out=w_sb["wvT"], in_=w_d["wvT"].rearrange("(k p) o -> p k o", p=128)
            )
            for nm in ("gamma", "beta"):
                nc.sync.dma_start(
                    out=b_sb[nm],
                    in_=b_d[nm].rearrange("(k p) one -> p (k one)", p=128),
                )
            gsel_sb = const.tile([128, 16], F32, tag="gsel")
            nc.sync.dma_start(out=gsel_sb, in_=gsel_d)
            gselT_sb = const.tile([16, 128], F32, tag="gselT")
            nc.sync.dma_start(out=gselT_sb, in_=gselT_d)
            nc.sync.dma_start(
                out=w_sb["woT"], in_=w_d["woT"].rearrange("(k p) o -> p k o", p=128)
            )

            # warm the Exp ACT table while input DMAs run
            eps_sb = const.tile([16, 1], F32, tag="eps")
            nc.vector.memset(eps_sb, EPS)
            warm_e = small.tile([16, 1], F32, tag="warme", bufs=1)
            nc.scalar.activation(out=warm_e, in_=eps_sb, func=AF.Exp)
            nc.sync.dma_start(out=warm2_d, in_=warm_e)

            q_sb = main.tile([128, 2, NHALF], F16, tag="q")
            q2_sb = main.tile([128, 2, NHALF], F16, tag="q2")
            k_sb = main.tile([128, 2, HW], F16, tag="k")
            k2_sb = main.tile([128, 2, HW], F16, tag="k2")
            vT_sb = main.tile([128, NJT, NH * (HD + 1)], F16, tag="vT")
            ao_sb = main.tile([128, 2, NHALF], F16, tag="ao")
            aon_sb = main.tile([128, 2, NIT, 128], F16, tag="aon")
            y_sb = main.tile([128, 2, NHALF], F32, tag="y")
            scr_sb = main.tile([128, NHALF], F16, tag="scr")

            # ones columns of vT (one per head, strided over j-tiles)
            for h in range(NH):
                c0 = h * (HD + 1) + HD
                nc.vector.memset(vT_sb[:, :, c0 : c0 + 1], 1.0)

            if USE_CWARM:
                # dummy tiny AllReduce overlapped with attention -- absorbs
                # first-use collective latency for the GN-tail AllReduce.
                cwarm_sb = small.tile([16, 1], F32, tag="cwarm", bufs=1)
                nc.vector.memset(cwarm_sb, 1.0)
                cw_in = dram.tile([16, 1], F32, tag="cwin", bufs=1)
                cw_out = dram.tile([16, 1], F32, tag="cwout", bufs=1)
                nc.sync.dma_start(out=cw_in, in_=cwarm_sb)
                nc.gpsimd.collective_compute(
                    "AllReduce", ALU.add,
                    replica_groups=[[0, 1], [2, 3], [4, 5], [6, 7]],
                    ins=[cw_in.opt()], outs=[cw_out.opt()],
                )

            # ---- projections + V in the pp PSUM pool (8 banks) ----
            with (
                tc.tile_pool(name="pp", bufs=2, space="PSUM") as pp,
            ):
                def proj(dst, wT, src, src_off, bias, nm):
                    """dst[0:1152] = wT.T @ src[src_off:+1152] + bias, via a
                    3-bank psum tile; copy+bias split ACT (first 512) / DVE."""
                    ps = pp.tile([128, NHALF], F32, tag="qk", name=f"ps_{nm}")
                    for kk in range(2):
                        for s, e in _mm_slices(NHALF):
                            nc.tensor.matmul(
                                ps[:, s:e], wT[:, kk, :],
                                src[:, kk, src_off + s : src_off + e],
                                start=(kk == 0), stop=(kk == 1),
                            )
                    nc.scalar.activation(out=dst[:, 0:512], in_=ps[:, 0:512],
                                         func=AF.Identity, bias=bias)
                    nc.vector.tensor_scalar_add(out=dst[:, 512:NHALF],
                                                in0=ps[:, 512:NHALF],
                                                scalar1=bias)

                for g in range(2):
                    proj(q_sb[:, g, :], w_sb["wqT"][:, :, g * 128 : (g + 1) * 128],
                         xh_sb, 0, b_sb["bq"][:, g : g + 1], f"q{g}")
                nc.sync.dma_start(out=q2_sb[0:64, :, :], in_=q_sb[64:128, :, :])
                nc.sync.dma_start(out=q2_sb[64:128, :, :], in_=q_sb[0:64, :, :])
                for jh in range(2):
                    for g in range(2):
                        proj(k_sb[:, g, jh * NHALF : (jh + 1) * NHALF],
                             w_sb["wkT"][:, :, g * 128 : (g + 1) * 128],
                             ctx_sb, jh * NHALF, b_sb["bk"][:, g : g + 1],
                             f"k{jh}{g}")
                    nc.sync.dma_start(
                        out=k2_sb[0:64, :, jh * NHALF : (jh + 1) * NHALF],
                        in_=k_sb[64:128, :, jh * NHALF : (jh + 1) * NHALF],
                    )
                    nc.sync.dma_start(
                        out=k2_sb[64:128, :, jh * NHALF : (jh + 1) * NHALF],
                        in_=k_sb[0:64, :, jh * NHALF : (jh + 1) * NHALF],
                    )

                # V: vT[j, c] = (ctx.T @ WvT)[j, c] per j-tile; copies split
                # across DVE / ACT by jt parity
                for jt in range(NJT):
                    vp = pp.tile([128, C], F32, tag="vp", name=f"vp{jt}")
                    for kk in range(2):
                        nc.tensor.matmul(
                            vp, ctx_sb[:, kk, jt * 128 : (jt + 1) * 128],
                            w_sb["wvT"][:, kk, :],
                            start=(kk == 0), stop=(kk == 1),
                        )
                    vdst = (
                        vT_sb[:, jt, :]
                        .rearrange("p (h e) -> p h e", e=HD + 1)[:, :, :HD]
                    )
                    vsrc = vp.rearrange("p (h d) -> p h d", d=HD)
                    if jt % 2 == 0:
                        nc.vector.tensor_copy(out=vdst, in_=vsrc)
                    else:
                        nc.scalar.activation(out=vdst, in_=vsrc, func=AF.Identity)

            # ---- attention: per head, AV trails scores/exp by 2 slots ----
            with (
                tc.tile_pool(name="expp", bufs=8) as expp,
                tc.tile_pool(name="scp", bufs=2, space="PSUM") as scp,
                tc.tile_pool(name="avp", bufs=1, space="PSUM") as avp,
            ):
                ex_tiles = {}
                av_banks = {}

                def issue_scores(h, jt):
                    # two PSUM tiles per slot so the ACT / DVE exp halves
                    # release their buffers independently:
                    #   sc_a [128,512] (1 bank):  q[0:512]    on rows 0-63
                    #   sc_b [128,640] (2 banks): q[512:1024] on rows 64-127
                    #                             q[1024:1152] on rows 0-63
                    g, ph = h // 2, h % 2
                    qlo, klo = (q_sb, k_sb) if ph == 0 else (q2_sb, k2_sb)
                    qhi, khi = (q2_sb, k2_sb) if ph == 0 else (q_sb, k_sb)
                    sca = scp.tile([128, 512], F32, tag="sca", name=f"sca{h}_{jt}")
                    scb = scp.tile([128, 640], F32, tag="scb", name=f"scb{h}_{jt}")
                    j0, j1 = jt * 128, (jt + 1) * 128
                    nc.tensor.matmul(
                        scb[:, 512:640], klo[0:64, g, j0:j1],
                        qlo[0:64, g, 1024:1152],
                        start=True, stop=True,
                    )
                    nc.tensor.matmul(
                        scb[:, 0:512], khi[64:128, g, j0:j1],
                        qhi[64:128, g, 512:1024],
                        start=True, stop=True,
                    )
                    nc.tensor.matmul(
                        sca[:, 0:512], klo[0:64, g, j0:j1],
                        qlo[0:64, g, 0:512],
                        start=True, stop=True,
                    )
                    return sca, scb

                def issue_exp(h, jt, sc):
                    sca, scb = sc
                    ex = expp.tile([128, NHALF], F16, tag="exp", name=f"ex{h}_{jt}")
                    nc.scalar.activation(out=ex[:, 0:512], in_=sca,
                                         func=AF.Exp, scale=SCALE)
                    if USE_DVE_EXP:
                        nc.vector.tensor_scalar(
                            out=ex[:, 512:NHALF].bitcast(I16), in0=scb,
                            scalar1=EXP_A, scalar2=EXP_B,
                            op0=ALU.mult, op1=ALU.add,
                        )
                    else:
                        nc.scalar.activation(out=ex[:, 512:NHALF], in_=scb,
                                             func=AF.Exp, scale=SCALE)
                    ex_tiles[h, jt] = ex

                def issue_av(h, jt):
                    ava, avb = av_banks[h]
                    col = h * (HD + 1)
                    for it in range(NIT):
                        dst = (
                            ava[:, it * (HD + 1) : (it + 1) * (HD + 1)]
                            if it < NA
                            else avb[:, (it - NA) * (HD + 1) : (it - NA + 1) * (HD + 1)]
                        )
                        nc.tensor.matmul(
                            dst,
                            ex_tiles[h, jt][:, it * 128 : (it + 1) * 128],
                            vT_sb[:, jt, col : col + HD + 1],
                            start=(jt == 0 and it in (0, NA)),
                            stop=(jt == NJT - 1),
                            skip_group_check=True,
                        )
                    del ex_tiles[h, jt]

                def issue_norm(h):
                    ava, avb = av_banks[h]
                    g, ph = h // 2, h % 2
                    rdena = small.tile([128, NA], F32, tag="rdena", bufs=2)
                    rdenb = small.tile([128, NIT - NA], F32, tag="rdenb", bufs=2)
                    nc.vector.reciprocal(out=rdena, in_=ava[:, HD :: HD + 1])
                    nc.vector.reciprocal(out=rdenb, in_=avb[:, HD :: HD + 1])
                    for it in range(NIT):
                        src = (
                            ava[:, it * (HD + 1) : it * (HD + 1) + HD]
                            if it < NA
                            else avb[:, (it - NA) * (HD + 1) : (it - NA) * (HD + 1) + HD]
                        )
                        rd = (
                            rdena[:, it : it + 1] if it < NA
                            else rdenb[:, it - NA : it - NA + 1]
                        )
                        nc.vector.tensor_scalar_mul(
                            out=aon_sb[:, g, it, ph * HD : (ph + 1) * HD],
                            in0=src, scalar1=rd,
                        )

                for h in range(NH):
                    av_banks[h] = (
                        avp.tile([128, NA * (HD + 1)], F32, tag="ava",
                                 name=f"ava{h}"),
                        avp.tile([128, (NIT - NA) * (HD + 1)], F32, tag="avb",
                                 name=f"avb{h}"),
                    )
                    for jt in range(NJT):
                        issue_exp(h, jt, issue_scores(h, jt))
                        if jt >= 2:
                            issue_av(h, jt - 2)
                    for jt in range(NJT - 2, NJT):
                        issue_av(h, jt)
                    issue_norm(h)
                    if h % 2 == 1:  # both heads of the g-half staged
                        nc.sync.dma_start(
                            out=ao_sb[:, h // 2, :]
                            .rearrange("p (t i) -> p t i", i=128),
                            in_=aon_sb[:, h // 2, :, :],
                            transpose=True,
                        )

            if dbg:
                nc.sync.dma_start(out=dbg_q.rearrange("(k p) i -> p k i", p=128), in_=q_sb)
                nc.sync.dma_start(out=dbg_k.rearrange("(k p) i -> p k i", p=128), in_=k_sb)
                nc.sync.dma_start(out=dbg_v, in_=vT_sb)
                nc.sync.dma_start(out=dbg_ao.rearrange("(k p) i -> p k i", p=128), in_=ao_sb)

            # Sqrt lives in a different ACT table set than Exp: warm the sqrt
            # table during the Wo/stats phase (input dep on the transposed ao
            # keeps the Tile scheduler from hoisting it into the exp stream).
            warm = small.tile([16, 1], F32, tag="warm", bufs=1)
            nc.scalar.sqrt(out=warm, in_=ao_sb[0:16, 1, 0:1])
            nc.sync.dma_start(out=warm_d, in_=warm)

            # ---- output projection + residual + GroupNorm ----
            with (
                tc.tile_pool(name="wop", bufs=2, space="PSUM") as wop,
                tc.tile_pool(name="gnp", bufs=1, space="PSUM") as gnp,
            ):
                st4 = small.tile([128, 2, 2], F32, tag="st4", bufs=1)
                wps = []
                for kk in range(2):
                    for g in range(2):
                        if kk == 0:
                            wps.append(wop.tile([128, NHALF], F32, tag="wo",
                                                name=f"wo{g}"))
                        lhsT = w_sb["woT"][:, kk, g * 128 : (g + 1) * 128]
                        for s, e in _mm_slices(NHALF):
                            nc.tensor.matmul(wps[g][:, s:e], lhsT,
                                             ao_sb[:, kk, s:e],
                                             start=(kk == 0), stop=(kk == 1))
                for g in range(2):
                    # y = (wo + bo) + x, channel sums ride in accum_out
                    nc.vector.scalar_tensor_tensor(
                        out=y_sb[:, g, :], in0=wps[g],
                        scalar=b_sb["bo"][:, g : g + 1],
                        in1=xh_sb[:, g, :],
                        op0=ALU.add, op1=ALU.add,
                        accum_out=st4[:, g, 0:1],
                    )
                    nc.scalar.activation(
                        out=scr_sb, in_=y_sb[:, g, :], func=AF.Square,
                        accum_out=st4[:, g, 1:2],
                    )

                # pair AllReduce of per-channel (sum, sumsq)
                gn_in = dram.tile([C, 2], F32, tag="gnin", bufs=1)
                gn_out = dram.tile([C, 2], F32, tag="gnout", bufs=1)
                nc.sync.dma_start(
                    out=gn_in.rearrange("(k p) s -> p k s", p=128), in_=st4
                )
                nc.gpsimd.collective_compute(
                    "AllReduce", ALU.add,
                    replica_groups=[[0, 1], [2, 3], [4, 5], [6, 7]],
                    ins=[gn_in.opt()], outs=[gn_out.opt()],
                )
                gs_sb = small.tile([128, 2, 2], F32, tag="gs", bufs=1)
                nc.sync.dma_start(
                    out=gs_sb, in_=gn_out.rearrange("(k p) s -> p k s", p=128)
                )

                # group totals via 0/1 selection matmul: [16, (sum,sumsq)]
                gtot = small.tile([16, 2, 2], F32, tag="gtot", bufs=1)
                for kk in range(2):
                    gp = gnp.tile([16, 2], F32, tag="gp")
                    nc.tensor.matmul(gp, gsel_sb, gs_sb[:, kk, :],
                                     start=True, stop=True)
                    nc.vector.tensor_copy(out=gtot[:, kk, :], in_=gp)

                mean_g = gtot[:, :, 0]
                var_g = small.tile([16, 2], F32, tag="varg", bufs=1)
                nc.vector.tensor_copy(out=var_g, in_=gtot[:, :, 1])
                m2 = small.tile([16, 2], F32, tag="m2", bufs=1)
                nc.vector.tensor_mul(out=m2, in0=mean_g, in1=mean_g)
                nc.vector.tensor_tensor(out=var_g, in0=var_g, in1=m2,
                                        op=ALU.subtract)
                nc.scalar.activation(out=var_g, in_=var_g, func=AF.Sqrt,
                                     bias=eps_sb)
                nc.vector.reciprocal(out=var_g, in_=var_g)  # rstd [16, 2]

                # broadcast group stats to channels, fold gamma/beta into A,B
                mean_c = small.tile([128, 2], F32, tag="meanc", bufs=1)
                rstd_c = small.tile([128, 2], F32, tag="rstdc", bufs=1)
                for src, dst in ((mean_g, mean_c), (var_g, rstd_c)):
                    gp = gnp.tile([128, 2], F32, tag="gb")
                    nc.tensor.matmul(gp, gselT_sb, src, start=True, stop=True)
                    nc.vector.tensor_copy(out=dst, in_=gp)
                a_c = small.tile([128, 2], F32, tag="ac", bufs=1)
                b_c = small.tile([128, 2], F32, tag="bc", bufs=1)
                nc.vector.tensor_mul(out=a_c, in0=rstd_c, in1=b_sb["gamma"])
                nc.vector.tensor_mul(out=b_c, in0=mean_c, in1=a_c)
                nc.vector.tensor_tensor(out=b_c, in0=b_sb["beta"], in1=b_c,
                                        op=ALU.subtract)

                if dbg:
                    nc.sync.dma_start(out=dbg_y.rearrange("(k p) i -> p k i", p=128), in_=y_sb)
                    nc.sync.dma_start(out=dbg_st.rearrange("(k p) s -> p k s", p=128)[:, :, 0:1], in_=st4[:, :, 0:1])
                    nc.sync.dma_start(out=dbg_st.rearrange("(k p) s -> p k s", p=128)[:, :, 1:2], in_=st4[:, :, 1:2])
                yr = yh_d.rearrange("(k p) i -> p k i", p=128)
                for g in range(2):
                    for ci, (s, e) in enumerate(_mm_slices(NHALF)):
                        nc.vector.tensor_scalar(
                            out=y_sb[:, g, s:e], in0=y_sb[:, g, s:e],
                            scalar1=a_c[:, g : g + 1], scalar2=b_c[:, g : g + 1],
                            op0=ALU.mult, op1=ALU.add,
                        )
                        nc.sync.dma_start(out=yr[:, g, s:e], in_=y_sb[:, g, s:e])

    _finalize(nc)
    return nc


def _get_nc(dbg=False):
    key = "ncd" if dbg else "nc"
    if key not in _CACHE:
        _CACHE[key] = _build(dbg)
    return _CACHE[key]


def make_in_maps(x, context, Wq, bq, Wk, bk, Wv, bv, Wo, bo, gamma, beta):
    x = np.asarray(x, np.float32)
    context = np.asarray(context, np.float32)
    xr = np.ascontiguousarray(x.reshape(B, C, HW).astype(np.float16))
    cr = np.ascontiguousarray(context.reshape(B, C, HW).astype(np.float16))

    gsel = np.zeros((128, 16), np.float32)
    gsel[np.arange(128), np.arange(128) // GSIZE] = 1.0 / GN_COUNT

    Wo32 = np.asarray(Wo, np.float32)
    bo_eff = np.asarray(bo, np.float32) + Wo32 @ np.asarray(bv, np.float32)

    shared = {
        "wqT": np.ascontiguousarray(np.asarray(Wq, np.float32).T.astype(np.float16)),
        "wkT": np.ascontiguousarray(np.asarray(Wk, np.float32).T.astype(np.float16)),
        "wvT": np.ascontiguousarray(np.asarray(Wv, np.float32).T.astype(np.float16)),
        "woT": np.ascontiguousarray(Wo32.T.astype(np.float16)),
        "bq": np.asarray(bq, np.float32).reshape(C, 1),
        "bk": np.asarray(bk, np.float32).reshape(C, 1),
        "bo": bo_eff.reshape(C, 1),
        "gamma": np.asarray(gamma, np.float32).reshape(C, 1),
        "beta": np.asarray(beta, np.float32).reshape(C, 1),
        "gsel": gsel,
        "gselT": np.ascontiguousarray((gsel.T != 0).astype(np.float32)),
    }
    in_maps = []
    for core in range(8):
        b, half = core // 2, core % 2
        m = dict(shared)
        m["xh"] = np.ascontiguousarray(xr[b, :, half * NHALF : (half + 1) * NHALF])
        m["ctx"] = cr[b]
        in_maps.append(m)
    return in_maps


def kernel(x, context, Wq, bq, Wk, bk, Wv, bv, Wo, bo, gamma, beta):
    in_maps = make_in_maps(
        x, context, Wq, bq, Wk, bk, Wv, bv, Wo, bo, gamma, beta
    )
    x = np.asarray(x, np.float32)

    nc = _get_nc()
    res = run_bass_kernel_spmd(nc, in_maps, core_ids=list(range(8)))

    out = np.empty((B, C, HW), np.float32)
    for core in range(8):
        b, half = core // 2, core % 2
        out[b, :, half * NHALF : (half + 1) * NHALF] = res.results[core]["yh"]
    return out.reshape(x.shape)
